# revision 9
# baseline (speedup 1.0000x reference)
"""Trainium2 Bass kernel for nn_CoreRelu_83863531422003 (5-layer MLP).

Network (per reference):
    h0 = relu(X @ W0 + b0)                      X:[N,512] W0:[512,1024]
    hk = relu(LN(h_{k-1} @ Wk) * gk + bek)      Wk:[1024,1024], k=1..3
    y  = relu(h3 @ Wout + bout)                 Wout:[1024,1]

Sharding: data-parallel over rows across 8 NeuronCores (8192 rows/core),
weights replicated. No communication.

On-chip layout ("option B", feature-major): activations live transposed in
SBUF as [feat(partition), rows(free)]; weights are the stationary matmul
operand so layer outputs stay feature-major and no per-layer transposes are
needed. Only X is transposed once (PE transpose) on entry.

LayerNorm: host pre-centers hidden weights (W_c = W - W.mean(axis=1)) so
z = h @ W_c is exactly mean-free; variance = sumsq(z)/1024, computed with a
ones[128x128]-stationary matmul over z^2 which also replicates the per-row
sum across all 128 partitions (needed for the feature-major apply).

Matmuls run as float32r (1 cycle/row for moving free dim >=256 vs 4 for
plain fp32).
"""

import numpy as np
from contextlib import ExitStack

import concourse.bass as bass
import concourse.bacc as bacc
import concourse.tile as tile
from concourse import mybir
from concourse.bass_utils import run_bass_kernel_spmd

N_CORES = 8
N_FULL = 65536
D_IN = 512
W_HID = 1024
ROWS = N_FULL // N_CORES      # 8192 rows per core
R = 512                       # rows per chunk (one PSUM bank of fp32)
NCHUNK = ROWS // R            # 16
KT0 = D_IN // 128             # 4 k-tiles for layer 0
KT = W_HID // 128             # 8 k-tiles for hidden layers
OT = W_HID // 128             # 8 output-feature tiles
LN_EPS = 1e-6

F32 = mybir.dt.float32
F32R = mybir.dt.float32r
MM_DT = F32R     # flip to F32 if fp32r numerics prove too loose

AF = mybir.ActivationFunctionType
ALU = mybir.AluOpType


def _r(ap):
    """Bitcast an AP to the matmul dtype."""
    if MM_DT == F32:
        return ap
    return ap.bitcast(MM_DT)


def build_nc(rows=ROWS):
    nchunk = rows // R
    nc = bacc.Bacc()

    x_d = nc.dram_tensor("x", [rows, D_IN], F32R, kind="ExternalInput")
    w0_d = nc.dram_tensor("w0s", [128, KT0 * OT * 128], F32R, kind="ExternalInput")
    wh_d = [
        nc.dram_tensor(f"w{k}s", [128, KT * OT * 128], F32R, kind="ExternalInput")
        for k in (1, 2, 3)
    ]
    wo_d = nc.dram_tensor("wos", [128, KT], F32R, kind="ExternalInput")
    b0_d = nc.dram_tensor("b0s", [128, OT], F32, kind="ExternalInput")
    g_d = [nc.dram_tensor(f"g{k}s", [128, OT], F32, kind="ExternalInput") for k in (1, 2, 3)]
    be_d = [nc.dram_tensor(f"be{k}s", [128, OT], F32, kind="ExternalInput") for k in (1, 2, 3)]
    bo_d = nc.dram_tensor("bo", [1, 1], F32, kind="ExternalInput")
    out_d = nc.dram_tensor("out", [rows, 1], F32, kind="ExternalOutput")

    ident_d = nc.dram_tensor("ident", [128, 128], F32R, kind="ExternalInput")
    ones_d = nc.dram_tensor("onesm", [128, 128], F32R, kind="ExternalInput")

    with tile.TileContext(nc) as tc, ExitStack() as ctx:
        const = ctx.enter_context(tc.tile_pool(name="const", bufs=1))
        p_xin = ctx.enter_context(tc.tile_pool(name="xin", bufs=4))
        p_xt = ctx.enter_context(tc.tile_pool(name="xt", bufs=2))
        p_h = ctx.enter_context(tc.tile_pool(name="h", bufs=2))
        p_zc = ctx.enter_context(tc.tile_pool(name="zc", bufs=8))
        p_zsq = ctx.enter_context(tc.tile_pool(name="zsq", bufs=2))
        p_u = ctx.enter_context(tc.tile_pool(name="u", bufs=2))
        p_sq = ctx.enter_context(tc.tile_pool(name="sq", bufs=2))
        p_rc = ctx.enter_context(tc.tile_pool(name="rc", bufs=2))
        p_ob = ctx.enter_context(tc.tile_pool(name="ob", bufs=2))
        ps_z = ctx.enter_context(tc.tile_pool(name="psz", bufs=5, space="PSUM"))
        ps_tp = ctx.enter_context(tc.tile_pool(name="pstp", bufs=2, space="PSUM"))
        ps_st = ctx.enter_context(tc.tile_pool(name="psst", bufs=1, space="PSUM"))

        # --- resident constants ---
        w0t = const.tile([128, KT0 * OT * 128], F32R)
        nc.sync.dma_start(w0t[:], w0_d[:])
        wht = []
        for k in range(3):
            t = const.tile([128, KT * OT * 128], F32R, tag=f"w{k + 1}t")
            nc.sync.dma_start(t[:], wh_d[k][:])
            wht.append(t)
        wot = const.tile([128, KT], F32R)
        nc.sync.dma_start(wot[:], wo_d[:])
        b0t = const.tile([128, OT], F32)
        nc.sync.dma_start(b0t[:], b0_d[:])
        gt, bet = [], []
        for k in range(3):
            g = const.tile([128, OT], F32, tag=f"g{k + 1}t")
            nc.sync.dma_start(g[:], g_d[k][:])
            gt.append(g)
            b = const.tile([128, OT], F32, tag=f"be{k + 1}t")
            nc.sync.dma_start(b[:], be_d[k][:])
            bet.append(b)
        bot = const.tile([1, 1], F32)
        nc.sync.dma_start(bot[:], bo_d[:])
        idt = const.tile([128, 128], F32R)
        nc.sync.dma_start(idt[:], ident_d[:])
        onest = const.tile([128, 128], F32R)
        nc.sync.dma_start(onest[:], ones_d[:])
        epst = const.tile([128, 1], F32)
        nc.vector.memset(epst[:], LN_EPS)

        w0v = w0t[:].rearrange("p (kt ot m) -> p kt ot m", kt=KT0, ot=OT)
        whv = [t[:].rearrange("p (kt ot m) -> p kt ot m", kt=KT, ot=OT) for t in wht]

        for c in range(nchunk):
            # ---- load X chunk and transpose to feature-major xT[feat, rows]
            xin = []
            for rg in range(4):
                t = p_xin.tile([128, D_IN], F32R, tag="xin")
                nc.sync.dma_start(t[:], x_d[c * R + rg * 128 : c * R + (rg + 1) * 128, :])
                xin.append(t)
            xt = p_xt.tile([128, KT0, R], F32R, tag="xt")
            for rg in range(4):
                for ft in range(KT0):
                    tp = ps_tp.tile([128, 128], F32, tag="tp")
                    nc.tensor.transpose(
                        _r(tp[:]), _r(xin[rg][:, ft * 128 : (ft + 1) * 128]), _r(idt[:])
                    )
                    nc.scalar.copy(xt[:, ft, rg * 128 : (rg + 1) * 128], tp[:])

            # ---- layer 0: h0 = relu(X @ W0 + b0)
            hprev = p_h.tile([128, KT, R], F32R, tag="h")
            for ot in range(OT):
                z = ps_z.tile([128, R], F32, tag="z")
                for kt in range(KT0):
                    nc.tensor.matmul(
                        z[:],
                        _r(w0v[:, kt, ot, :]),
                        _r(xt[:, kt, :]),
                        start=(kt == 0),
                        stop=(kt == KT0 - 1),
                    )
                nc.scalar.activation(
                    hprev[:, ot, :], z[:], AF.Relu, bias=b0t[:, ot : ot + 1], scale=1.0
                )

            # ---- hidden layers 1..3: h = relu(LN(h @ Wc) * g + be)
            for k in range(3):
                hn = p_h.tile([128, KT, R], F32R, tag="h")
                zcs = []
                zsqs = []
                for ot in range(OT):
                    z = ps_z.tile([128, R], F32, tag="z")
                    for kt in range(KT):
                        nc.tensor.matmul(
                            z[:],
                            _r(whv[k][:, kt, ot, :]),
                            _r(hprev[:, kt, :]),
                            start=(kt == 0),
                            stop=(kt == KT - 1),
                        )
                    zc = p_zc.tile([128, R], F32, tag="zc")
                    nc.scalar.copy(zc[:], z[:])
                    zsq = p_zsq.tile([128, R], F32R, tag="zsq")
                    nc.vector.tensor_mul(zsq[:], zc[:], zc[:])
                    zcs.append(zc)
                    zsqs.append(zsq)
                st = ps_st.tile([128, R], F32, tag="st")
                for ot in range(OT):
                    nc.tensor.matmul(
                        st[:],
                        _r(onest[:]),
                        _r(zsqs[ot][:]),
                        start=(ot == 0),
                        stop=(ot == OT - 1),
                        skip_group_check=True,
                    )
                # sqrt(var + eps), var = sumsq / 1024 ; then 1/sqrt on DVE
                sq = p_sq.tile([128, R], F32, tag="sq")
                nc.scalar.activation(
                    sq[:], st[:], AF.Sqrt, bias=epst[:], scale=1.0 / W_HID
                )
                rc = p_rc.tile([128, R], F32, tag="rc")
                nc.vector.reciprocal(rc[:], sq[:])
                for ot in range(OT):
                    u = p_u.tile([128, R], F32, tag="u")
                    nc.vector.scalar_tensor_tensor(
                        u[:], zcs[ot][:], gt[k][:, ot : ot + 1], rc[:],
                        op0=ALU.mult, op1=ALU.mult,
                    )
                    nc.scalar.activation(
                        hn[:, ot, :], u[:], AF.Relu, bias=bet[k][:, ot : ot + 1],
                        scale=1.0,
                    )
                hprev = hn

            # ---- output layer: y = relu(h3 @ Wout + bout)
            zo = ps_st.tile([128, R], F32, tag="st")
            for kt in range(KT):
                nc.tensor.matmul(
                    zo[:1, :],
                    _r(wot[:, kt : kt + 1]),
                    _r(hprev[:, kt, :]),
                    start=(kt == 0),
                    stop=(kt == KT - 1),
                )
            ob = p_ob.tile([1, R], F32, tag="ob")
            nc.scalar.activation(ob[:], zo[:1, :], AF.Relu, bias=bot[:, :], scale=1.0)
            nc.sync.dma_start(out_d[c * R : (c + 1) * R, :], ob[:])

    nc.finalize()
    return nc


def _prep_inputs(inputs):
    """Host-side weight repack (float64 intermediates for the centering)."""
    f32 = np.float32

    def center(w):
        w64 = np.asarray(w, np.float64)
        return (w64 - w64.mean(axis=1, keepdims=True)).astype(f32)

    def pack_w(w, kt):
        # [kt*128, 1024] -> sbuf layout [p, kt, ot, m] flattened
        return (
            np.ascontiguousarray(
                np.asarray(w, f32).reshape(kt, 128, OT, 128).transpose(1, 0, 2, 3)
            ).reshape(128, kt * OT * 128)
        )

    def pack_v(v):
        # [1024] -> [128, OT] with [:, ot] = v[ot*128:(ot+1)*128]
        return np.ascontiguousarray(np.asarray(v, f32).reshape(OT, 128).T)

    common = {
        "w0s": pack_w(inputs["W0"], KT0),
        "w1s": pack_w(center(inputs["W1"]), KT),
        "w2s": pack_w(center(inputs["W2"]), KT),
        "w3s": pack_w(center(inputs["W3"]), KT),
        "wos": np.ascontiguousarray(np.asarray(inputs["Wout"], f32).reshape(KT, 128).T),
        "b0s": pack_v(inputs["b0"]),
        "g1s": pack_v(inputs["g1"]),
        "be1s": pack_v(inputs["be1"]),
        "g2s": pack_v(inputs["g2"]),
        "be2s": pack_v(inputs["be2"]),
        "g3s": pack_v(inputs["g3"]),
        "be3s": pack_v(inputs["be3"]),
        "bo": np.asarray(inputs["bout"], f32).reshape(1, 1),
        "ident": np.eye(128, dtype=f32),
        "onesm": np.ones((128, 128), dtype=f32),
    }
    return common


_NC_CACHE = {}


def _get_nc():
    if "nc" not in _NC_CACHE:
        _NC_CACHE["nc"] = build_nc()
    return _NC_CACHE["nc"]


def _run(inputs, trace=False):
    common = _prep_inputs(inputs)
    x = np.ascontiguousarray(np.asarray(inputs["descriptors"], np.float32))
    shards = x.reshape(N_CORES, ROWS, D_IN)
    in_maps = [dict(common, x=np.ascontiguousarray(shards[i])) for i in range(N_CORES)]
    nc = _get_nc()
    res = run_bass_kernel_spmd(nc, in_maps, core_ids=list(range(N_CORES)), trace=trace)
    out = np.concatenate([res.results[i]["out"] for i in range(N_CORES)], axis=0)
    return out.astype(np.float32), res


def kernel(**inputs):
    out, _ = _run(inputs, trace=False)
    return out


def kernel_traced(**inputs):
    out, res = _run(inputs, trace=True)
    return out, res


# revision 12
# speedup vs baseline: 1.3802x; 1.3802x over previous
"""Trainium2 Bass kernel for nn_CoreRelu_83863531422003 (5-layer MLP).

Network (per reference):
    h0 = relu(X @ W0 + b0)                      X:[N,512] W0:[512,1024]
    hk = relu(LN(h_{k-1} @ Wk) * gk + bek)      Wk:[1024,1024], k=1..3
    y  = relu(h3 @ Wout + bout)                 Wout:[1024,1]

Sharding: data-parallel over rows across 8 NeuronCores (8192 rows/core),
weights replicated. No communication.

On-chip layout ("option B", feature-major): activations live transposed in
SBUF as [feat(partition), rows(free)]; weights are the stationary matmul
operand so layer outputs stay feature-major and no per-layer transposes are
needed. Only X is transposed once (PE transpose) on entry.

LayerNorm: host pre-centers hidden weights (W_c = W - W.mean(axis=1)) so
z = h @ W_c is exactly mean-free; variance = sumsq(z)/1024, computed with a
ones[128x128]-stationary matmul over z^2 which also replicates the per-row
sum across all 128 partitions (needed for the feature-major apply).

Matmuls run as float32r (1 cycle/row for moving free dim >=256 vs 4 for
plain fp32).
"""

import numpy as np
from contextlib import ExitStack

import concourse.bass as bass
import concourse.bacc as bacc
import concourse.tile as tile
from concourse import mybir
from concourse.bass_utils import run_bass_kernel_spmd

N_CORES = 8
N_FULL = 65536
D_IN = 512
W_HID = 1024
ROWS = N_FULL // N_CORES      # 8192 rows per core
R = 512                       # rows per chunk (one PSUM bank of fp32)
NCHUNK = ROWS // R            # 16
KT0 = D_IN // 128             # 4 k-tiles for layer 0
KT = W_HID // 128             # 8 k-tiles for hidden layers
OT = W_HID // 128             # 8 output-feature tiles
LN_EPS = 1e-6

F32 = mybir.dt.float32
F32R = mybir.dt.float32r
MM_DT = F32R     # flip to F32 if fp32r numerics prove too loose

AF = mybir.ActivationFunctionType
ALU = mybir.AluOpType


def _r(ap):
    """Bitcast an AP to the matmul dtype."""
    if MM_DT == F32:
        return ap
    return ap.bitcast(MM_DT)


def build_nc(rows=ROWS):
    nchunk = rows // R
    nc = bacc.Bacc()

    x_d = nc.dram_tensor("x", [rows, D_IN], F32R, kind="ExternalInput")
    w0_d = nc.dram_tensor("w0s", [128, KT0 * OT * 128], F32R, kind="ExternalInput")
    wh_d = [
        nc.dram_tensor(f"w{k}s", [128, KT * OT * 128], F32R, kind="ExternalInput")
        for k in (1, 2, 3)
    ]
    wo_d = nc.dram_tensor("wos", [128, KT], F32R, kind="ExternalInput")
    b0_d = nc.dram_tensor("b0s", [128, OT], F32, kind="ExternalInput")
    g_d = [nc.dram_tensor(f"g{k}s", [128, OT], F32, kind="ExternalInput") for k in (1, 2, 3)]
    be_d = [nc.dram_tensor(f"be{k}s", [128, OT], F32, kind="ExternalInput") for k in (1, 2, 3)]
    bo_d = nc.dram_tensor("bo", [1, 1], F32, kind="ExternalInput")
    out_d = nc.dram_tensor("out", [rows, 1], F32, kind="ExternalOutput")

    ident_d = nc.dram_tensor("ident", [128, 128], F32R, kind="ExternalInput")
    ones_d = nc.dram_tensor("onesm", [128, 128], F32R, kind="ExternalInput")

    with tile.TileContext(nc) as tc, ExitStack() as ctx:
        const = ctx.enter_context(tc.tile_pool(name="const", bufs=1))
        p_xin = ctx.enter_context(tc.tile_pool(name="xin", bufs=4))
        p_xt = ctx.enter_context(tc.tile_pool(name="xt", bufs=2))
        p_h = ctx.enter_context(tc.tile_pool(name="h", bufs=2))
        p_zc = ctx.enter_context(tc.tile_pool(name="zc", bufs=8))
        p_zsq = ctx.enter_context(tc.tile_pool(name="zsq", bufs=2))
        p_u = ctx.enter_context(tc.tile_pool(name="u", bufs=2))
        p_sq = ctx.enter_context(tc.tile_pool(name="sq", bufs=2))
        p_rc = ctx.enter_context(tc.tile_pool(name="rc", bufs=2))
        p_ob = ctx.enter_context(tc.tile_pool(name="ob", bufs=2))
        ps_z = ctx.enter_context(tc.tile_pool(name="psz", bufs=5, space="PSUM"))
        ps_tp = ctx.enter_context(tc.tile_pool(name="pstp", bufs=2, space="PSUM"))
        ps_st = ctx.enter_context(tc.tile_pool(name="psst", bufs=1, space="PSUM"))

        # --- resident constants ---
        w0t = const.tile([128, KT0 * OT * 128], F32R)
        nc.sync.dma_start(w0t[:], w0_d[:])
        wht = []
        for k in range(3):
            t = const.tile([128, KT * OT * 128], F32R, tag=f"w{k + 1}t")
            nc.sync.dma_start(t[:], wh_d[k][:])
            wht.append(t)
        wot = const.tile([128, KT], F32R)
        nc.sync.dma_start(wot[:], wo_d[:])
        b0t = const.tile([128, OT], F32)
        nc.sync.dma_start(b0t[:], b0_d[:])
        gt, bet = [], []
        for k in range(3):
            g = const.tile([128, OT], F32, tag=f"g{k + 1}t")
            nc.sync.dma_start(g[:], g_d[k][:])
            gt.append(g)
            b = const.tile([128, OT], F32, tag=f"be{k + 1}t")
            nc.sync.dma_start(b[:], be_d[k][:])
            bet.append(b)
        bot = const.tile([1, 1], F32)
        nc.sync.dma_start(bot[:], bo_d[:])
        idt = const.tile([128, 128], F32R)
        nc.sync.dma_start(idt[:], ident_d[:])
        onest = const.tile([128, 128], F32R)
        nc.sync.dma_start(onest[:], ones_d[:])
        epst = const.tile([128, 1], F32)
        nc.vector.memset(epst[:], LN_EPS)

        w0v = w0t[:].rearrange("p (kt ot m) -> p kt ot m", kt=KT0, ot=OT)
        whv = [t[:].rearrange("p (kt ot m) -> p kt ot m", kt=KT, ot=OT) for t in wht]

        for c in range(nchunk):
            # ---- load X chunk and transpose to feature-major xT[feat, rows]
            xin = []
            for rg in range(4):
                t = p_xin.tile([128, D_IN], F32R, tag="xin")
                nc.sync.dma_start(t[:], x_d[c * R + rg * 128 : c * R + (rg + 1) * 128, :])
                xin.append(t)
            xt = p_xt.tile([128, KT0, R], F32R, tag="xt")
            for rg in range(4):
                for ft in range(KT0):
                    tp = ps_tp.tile([128, 128], F32, tag="tp")
                    nc.tensor.transpose(
                        _r(tp[:]), _r(xin[rg][:, ft * 128 : (ft + 1) * 128]), _r(idt[:])
                    )
                    nc.scalar.copy(xt[:, ft, rg * 128 : (rg + 1) * 128], tp[:])

            # ---- layer 0: h0 = relu(X @ W0 + b0)
            hprev = p_h.tile([128, KT, R], F32R, tag="h")
            for ot in range(OT):
                z = ps_z.tile([128, R], F32, tag="z")
                for kt in range(KT0):
                    nc.tensor.matmul(
                        z[:],
                        _r(w0v[:, kt, ot, :]),
                        _r(xt[:, kt, :]),
                        start=(kt == 0),
                        stop=(kt == KT0 - 1),
                    )
                nc.scalar.activation(
                    hprev[:, ot, :], z[:], AF.Relu, bias=b0t[:, ot : ot + 1], scale=1.0
                )

            # ---- hidden layers 1..3: h = relu(LN(h @ Wc) * g + be)
            for k in range(3):
                hn = p_h.tile([128, KT, R], F32R, tag="h")
                zcs = []
                zsqs = []
                for ot in range(OT):
                    z = ps_z.tile([128, R], F32, tag="z")
                    for kt in range(KT):
                        nc.tensor.matmul(
                            z[:],
                            _r(whv[k][:, kt, ot, :]),
                            _r(hprev[:, kt, :]),
                            start=(kt == 0),
                            stop=(kt == KT - 1),
                        )
                    zc = p_zc.tile([128, R], F32, tag="zc")
                    nc.scalar.copy(zc[:], z[:])
                    zsq = p_zsq.tile([128, R], F32R, tag="zsq")
                    nc.vector.tensor_mul(zsq[:], zc[:], zc[:])
                    zcs.append(zc)
                    zsqs.append(zsq)
                st = ps_st.tile([128, R], F32, tag="st")
                for ot in range(OT):
                    nc.tensor.matmul(
                        st[:],
                        _r(onest[:]),
                        _r(zsqs[ot][:]),
                        start=(ot == 0),
                        stop=(ot == OT - 1),
                        skip_group_check=True,
                    )
                # sqrt(var + eps), var = sumsq / 1024 ; then 1/sqrt on DVE
                sq = p_sq.tile([128, R], F32, tag="sq")
                nc.scalar.activation(
                    sq[:], st[:], AF.Sqrt, bias=epst[:], scale=1.0 / W_HID
                )
                rc = p_rc.tile([128, R], F32, tag="rc")
                nc.vector.reciprocal(rc[:], sq[:])
                for ot in range(OT):
                    u = p_u.tile([128, R], F32, tag="u")
                    nc.vector.scalar_tensor_tensor(
                        u[:], zcs[ot][:], gt[k][:, ot : ot + 1], rc[:],
                        op0=ALU.mult, op1=ALU.mult,
                    )
                    nc.scalar.activation(
                        hn[:, ot, :], u[:], AF.Relu, bias=bet[k][:, ot : ot + 1],
                        scale=1.0,
                    )
                hprev = hn

            # ---- output layer: y = relu(h3 @ Wout + bout)
            zo = ps_st.tile([128, R], F32, tag="st")
            for kt in range(KT):
                nc.tensor.matmul(
                    zo[:1, :],
                    _r(wot[:, kt : kt + 1]),
                    _r(hprev[:, kt, :]),
                    start=(kt == 0),
                    stop=(kt == KT - 1),
                )
            ob = p_ob.tile([1, R], F32, tag="ob")
            nc.scalar.activation(ob[:], zo[:1, :], AF.Relu, bias=bot[:, :], scale=1.0)
            nc.sync.dma_start(out_d[c * R : (c + 1) * R, :], ob[:])

    nc.finalize()
    return nc


def build_nc_fast(rows=ROWS):
    """Fast variant, valid when be1..be3 == 0 and g1..g3 > 0 elementwise.

    Uses fp16 matmul operands (1 cycle/row on the PE vs ~2 for fp32r) and
    defers the LayerNorm scaling: LN is invariant to positive per-row scaling
    of its input, and relu commutes with positive per-row scales, so each
    hidden layer just passes h~ = relu(z_c * g) forward unnormalized. The
    cumulative squared scale follows d2_k = m~_k + eps * d2_{k-1} (m~_k =
    weighted mean of z~^2 via a (1/(1024 g^2))-stationary matmul), and a
    single rsqrt per chunk rescales the output-layer logits.
    """
    nchunk = rows // R
    F16 = mybir.dt.float16
    nc = bacc.Bacc()

    x_d = nc.dram_tensor("x", [rows, D_IN], F16, kind="ExternalInput")
    w0_d = nc.dram_tensor("w0s", [128, KT0 * OT * 128], F16, kind="ExternalInput")
    wh_d = [
        nc.dram_tensor(f"w{k}s", [128, KT * OT * 128], F16, kind="ExternalInput")
        for k in (1, 2, 3)
    ]
    sw_d = [
        nc.dram_tensor(f"sw{k}s", [128, OT * 128], F16, kind="ExternalInput")
        for k in (1, 2, 3)
    ]
    wo_d = nc.dram_tensor("wos", [128, KT], F16, kind="ExternalInput")
    b0_d = nc.dram_tensor("b0s", [128, OT], F32, kind="ExternalInput")
    bo_d = nc.dram_tensor("bo", [1, 1], F32, kind="ExternalInput")
    ident_d = nc.dram_tensor("ident", [128, 128], F16, kind="ExternalInput")
    out_d = nc.dram_tensor("out", [rows, 1], F32, kind="ExternalOutput")

    with tile.TileContext(nc) as tc, ExitStack() as ctx:
        const = ctx.enter_context(tc.tile_pool(name="const", bufs=1))
        p_xin = ctx.enter_context(tc.tile_pool(name="xin", bufs=8))
        p_xt = ctx.enter_context(tc.tile_pool(name="xt", bufs=3))
        p_h = ctx.enter_context(tc.tile_pool(name="h", bufs=3))
        p_zsq = ctx.enter_context(tc.tile_pool(name="zsq", bufs=3))
        p_d2 = ctx.enter_context(tc.tile_pool(name="d2", bufs=4))
        p_s = ctx.enter_context(tc.tile_pool(name="s", bufs=2))
        p_ob = ctx.enter_context(tc.tile_pool(name="ob", bufs=3))
        ps_z = ctx.enter_context(tc.tile_pool(name="psz", bufs=4, space="PSUM"))
        ps_tp = ctx.enter_context(tc.tile_pool(name="pstp", bufs=2, space="PSUM"))
        ps_st = ctx.enter_context(tc.tile_pool(name="psst", bufs=1, space="PSUM"))
        ps_zo = ctx.enter_context(tc.tile_pool(name="pszo", bufs=1, space="PSUM"))

        # --- resident constants ---
        w0t = const.tile([128, KT0 * OT * 128], F16)
        nc.sync.dma_start(w0t[:], w0_d[:])
        wht = []
        swt = []
        for k in range(3):
            t = const.tile([128, KT * OT * 128], F16, tag=f"w{k + 1}t")
            nc.sync.dma_start(t[:], wh_d[k][:])
            wht.append(t)
            t = const.tile([128, OT, 128], F16, tag=f"sw{k + 1}t")
            nc.sync.dma_start(t[:], sw_d[k][:])
            swt.append(t)
        wot = const.tile([128, KT], F16)
        nc.sync.dma_start(wot[:], wo_d[:])
        b0t = const.tile([128, OT], F32)
        nc.sync.dma_start(b0t[:], b0_d[:])
        bot = const.tile([1, 1], F32)
        nc.sync.dma_start(bot[:], bo_d[:])
        idt = const.tile([128, 128], F16)
        nc.sync.dma_start(idt[:], ident_d[:])
        epst = const.tile([128, 1], F32)
        nc.vector.memset(epst[:], LN_EPS)
        zerot = const.tile([128, 1], F32)
        nc.vector.memset(zerot[:], 0.0)

        w0v = w0t[:].rearrange("p (kt ot m) -> p kt ot m", kt=KT0, ot=OT)
        whv = [t[:].rearrange("p (kt ot m) -> p kt ot m", kt=KT, ot=OT) for t in wht]

        for c in range(nchunk):
            # ---- load X chunk and transpose to feature-major xT[feat, rows]
            xin = []
            for rg in range(4):
                t = p_xin.tile([128, D_IN], F16, tag="xin")
                nc.sync.dma_start(t[:], x_d[c * R + rg * 128 : c * R + (rg + 1) * 128, :])
                xin.append(t)
            xt = p_xt.tile([128, KT0, R], F16, tag="xt")
            for rg in range(4):
                for ft in range(KT0):
                    tp = ps_tp.tile([128, 128], F16, tag="tp")
                    nc.tensor.transpose(
                        tp[:], xin[rg][:, ft * 128 : (ft + 1) * 128], idt[:]
                    )
                    nc.vector.tensor_copy(xt[:, ft, rg * 128 : (rg + 1) * 128], tp[:])

            # ---- layer 0: h0 = relu(X @ W0 + b0)
            hprev = p_h.tile([128, KT, R], F16, tag="h")
            for ot in range(OT):
                z = ps_z.tile([128, R], F32, tag="z")
                for kt in range(KT0):
                    nc.tensor.matmul(
                        z[:], w0v[:, kt, ot, :], xt[:, kt, :],
                        start=(kt == 0), stop=(kt == KT0 - 1),
                    )
                nc.scalar.activation(
                    hprev[:, ot, :], z[:], AF.Relu, bias=b0t[:, ot : ot + 1], scale=1.0
                )

            # ---- hidden layers: h~ = relu(h~prev @ (Wc*g)); m~ accumulated on PE
            d2 = None
            for k in range(3):
                hn = p_h.tile([128, KT, R], F16, tag="h")
                st = ps_st.tile([128, R], F32, tag="st")
                for ot in range(OT):
                    z = ps_z.tile([128, R], F32, tag="z")
                    for kt in range(KT):
                        nc.tensor.matmul(
                            z[:], whv[k][:, kt, ot, :], hprev[:, kt, :],
                            start=(kt == 0), stop=(kt == KT - 1),
                        )
                    nc.scalar.activation(
                        hn[:, ot, :], z[:], AF.Relu, bias=zerot[:], scale=1.0
                    )
                    zsq = p_zsq.tile([128, R], F16, tag="zsq")
                    nc.scalar.activation(
                        zsq[:], z[:], AF.Square, bias=zerot[:], scale=1.0
                    )
                    nc.tensor.matmul(
                        st[:], swt[k][:, ot, :], zsq[:],
                        start=(ot == 0), stop=(ot == OT - 1),
                        skip_group_check=True,
                    )
                # d2_k = m~_k + eps * d2_{k-1}
                d2n = p_d2.tile([128, R], F32, tag="d2")
                if d2 is None:
                    nc.scalar.activation(
                        d2n[:], st[:], AF.Identity, bias=epst[:], scale=1.0
                    )
                else:
                    nc.vector.scalar_tensor_tensor(
                        d2n[:], d2[:], LN_EPS, st[:], op0=ALU.mult, op1=ALU.add
                    )
                d2 = d2n
                hprev = hn

            # ---- output layer: y = relu((h3 @ Wout) * rsqrt(d2_3) + bout)
            zo = ps_zo.tile([128, R], F32, tag="zo")
            for kt in range(KT):
                nc.tensor.matmul(
                    zo[:1, :], wot[:, kt : kt + 1], hprev[:, kt, :],
                    start=(kt == 0), stop=(kt == KT - 1),
                )
            s = p_s.tile([128, R], F32, tag="s")
            nc.scalar.activation(s[:], d2[:], AF.Sqrt, bias=zerot[:], scale=1.0)
            rs = p_s.tile([128, R], F32, tag="rs")
            nc.vector.reciprocal(rs[:1, :], s[:1, :])
            yv = p_ob.tile([1, R], F32, tag="yv")
            nc.vector.tensor_mul(yv[:], zo[:1, :], rs[:1, :])
            ob = p_ob.tile([1, R], F32, tag="ob")
            nc.vector.tensor_scalar(
                ob[:], yv[:], bot[:, :], 0.0, op0=ALU.add, op1=ALU.max
            )
            nc.sync.dma_start(out_d[c * R : (c + 1) * R, :], ob[:])

    nc.finalize()
    return nc


def _prep_inputs(inputs):
    """Host-side weight repack (float64 intermediates for the centering)."""
    f32 = np.float32

    def center(w):
        w64 = np.asarray(w, np.float64)
        return (w64 - w64.mean(axis=1, keepdims=True)).astype(f32)

    def pack_w(w, kt):
        # [kt*128, 1024] -> sbuf layout [p, kt, ot, m] flattened
        return (
            np.ascontiguousarray(
                np.asarray(w, f32).reshape(kt, 128, OT, 128).transpose(1, 0, 2, 3)
            ).reshape(128, kt * OT * 128)
        )

    def pack_v(v):
        # [1024] -> [128, OT] with [:, ot] = v[ot*128:(ot+1)*128]
        return np.ascontiguousarray(np.asarray(v, f32).reshape(OT, 128).T)

    common = {
        "w0s": pack_w(inputs["W0"], KT0),
        "w1s": pack_w(center(inputs["W1"]), KT),
        "w2s": pack_w(center(inputs["W2"]), KT),
        "w3s": pack_w(center(inputs["W3"]), KT),
        "wos": np.ascontiguousarray(np.asarray(inputs["Wout"], f32).reshape(KT, 128).T),
        "b0s": pack_v(inputs["b0"]),
        "g1s": pack_v(inputs["g1"]),
        "be1s": pack_v(inputs["be1"]),
        "g2s": pack_v(inputs["g2"]),
        "be2s": pack_v(inputs["be2"]),
        "g3s": pack_v(inputs["g3"]),
        "be3s": pack_v(inputs["be3"]),
        "bo": np.asarray(inputs["bout"], f32).reshape(1, 1),
        "ident": np.eye(128, dtype=f32),
        "onesm": np.ones((128, 128), dtype=f32),
    }
    return common


def _prep_inputs_fast(inputs):
    """Host prep for the fast (be==0, g>0) variant: fp16 weights, g folded
    into the hidden weights, 1/(1024 g^2) folded into the stats stationary."""
    f16 = np.float16

    def center(w):
        w64 = np.asarray(w, np.float64)
        return w64 - w64.mean(axis=1, keepdims=True)

    def pack_w(w64, kt):
        return np.ascontiguousarray(
            w64.reshape(kt, 128, OT, 128).transpose(1, 0, 2, 3)
        ).reshape(128, kt * OT * 128).astype(f16)

    def stats_w(g):
        g64 = np.asarray(g, np.float64)
        vals = (1.0 / (W_HID * g64 * g64)).reshape(OT, 128)  # [ot, p]
        return np.ascontiguousarray(
            np.broadcast_to(vals.T[:, :, None], (128, OT, 128))
        ).reshape(128, OT * 128).astype(f16)

    common = {
        "w0s": pack_w(np.asarray(inputs["W0"], np.float64), KT0),
        "wos": np.ascontiguousarray(
            np.asarray(inputs["Wout"], np.float64).reshape(KT, 128).T
        ).astype(f16),
        "b0s": np.ascontiguousarray(
            np.asarray(inputs["b0"], np.float32).reshape(OT, 128).T
        ),
        "bo": np.asarray(inputs["bout"], np.float32).reshape(1, 1),
        "ident": np.eye(128, dtype=f16),
    }
    for k in (1, 2, 3):
        g64 = np.asarray(inputs[f"g{k}"], np.float64)
        common[f"w{k}s"] = pack_w(center(inputs[f"W{k}"]) * g64[None, :], KT)
        common[f"sw{k}s"] = stats_w(g64)
    return common


_NC_CACHE = {}


def _get_nc(fast):
    key = "fast" if fast else "general"
    if key not in _NC_CACHE:
        _NC_CACHE[key] = build_nc_fast() if fast else build_nc()
    return _NC_CACHE[key]


def _is_fast_ok(inputs):
    return all(
        np.all(np.asarray(inputs[f"be{k}"]) == 0)
        and np.all(np.asarray(inputs[f"g{k}"]) > 0)
        for k in (1, 2, 3)
    )


def _run(inputs, trace=False):
    fast = _is_fast_ok(inputs)
    common = _prep_inputs_fast(inputs) if fast else _prep_inputs(inputs)
    xdt = np.float16 if fast else np.float32
    x = np.ascontiguousarray(np.asarray(inputs["descriptors"], np.float32).astype(xdt))
    shards = x.reshape(N_CORES, ROWS, D_IN)
    in_maps = [dict(common, x=np.ascontiguousarray(shards[i])) for i in range(N_CORES)]
    nc = _get_nc(fast)
    res = run_bass_kernel_spmd(nc, in_maps, core_ids=list(range(N_CORES)), trace=trace)
    out = np.concatenate([res.results[i]["out"] for i in range(N_CORES)], axis=0)
    return out.astype(np.float32), res


def kernel(**inputs):
    out, _ = _run(inputs, trace=False)
    return out


def kernel_traced(**inputs):
    out, res = _run(inputs, trace=True)
    return out, res


# revision 14
# speedup vs baseline: 1.5474x; 1.1211x over previous
"""Trainium2 Bass kernel for nn_CoreRelu_83863531422003 (5-layer MLP).

Network (per reference):
    h0 = relu(X @ W0 + b0)                      X:[N,512] W0:[512,1024]
    hk = relu(LN(h_{k-1} @ Wk) * gk + bek)      Wk:[1024,1024], k=1..3
    y  = relu(h3 @ Wout + bout)                 Wout:[1024,1]

Sharding: data-parallel over rows across 8 NeuronCores (8192 rows/core),
weights replicated. No communication.

On-chip layout ("option B", feature-major): activations live transposed in
SBUF as [feat(partition), rows(free)]; weights are the stationary matmul
operand so layer outputs stay feature-major and no per-layer transposes are
needed. Only X is transposed once (PE transpose) on entry.

LayerNorm: host pre-centers hidden weights (W_c = W - W.mean(axis=1)) so
z = h @ W_c is exactly mean-free; variance = sumsq(z)/1024, computed with a
ones[128x128]-stationary matmul over z^2 which also replicates the per-row
sum across all 128 partitions (needed for the feature-major apply).

Matmuls run as float32r (1 cycle/row for moving free dim >=256 vs 4 for
plain fp32).
"""

import numpy as np
from contextlib import ExitStack

import concourse.bass as bass
import concourse.bacc as bacc
import concourse.tile as tile
from concourse import mybir
from concourse.bass_utils import run_bass_kernel_spmd

N_CORES = 8
N_FULL = 65536
D_IN = 512
W_HID = 1024
ROWS = N_FULL // N_CORES      # 8192 rows per core
R = 512                       # rows per chunk (one PSUM bank of fp32)
NCHUNK = ROWS // R            # 16
KT0 = D_IN // 128             # 4 k-tiles for layer 0
KT = W_HID // 128             # 8 k-tiles for hidden layers
OT = W_HID // 128             # 8 output-feature tiles
LN_EPS = 1e-6

F32 = mybir.dt.float32
F32R = mybir.dt.float32r
MM_DT = F32R     # flip to F32 if fp32r numerics prove too loose

AF = mybir.ActivationFunctionType
ALU = mybir.AluOpType


def _r(ap):
    """Bitcast an AP to the matmul dtype."""
    if MM_DT == F32:
        return ap
    return ap.bitcast(MM_DT)


def build_nc(rows=ROWS):
    nchunk = rows // R
    nc = bacc.Bacc()

    x_d = nc.dram_tensor("x", [rows, D_IN], F32R, kind="ExternalInput")
    w0_d = nc.dram_tensor("w0s", [128, KT0 * OT * 128], F32R, kind="ExternalInput")
    wh_d = [
        nc.dram_tensor(f"w{k}s", [128, KT * OT * 128], F32R, kind="ExternalInput")
        for k in (1, 2, 3)
    ]
    wo_d = nc.dram_tensor("wos", [128, KT], F32R, kind="ExternalInput")
    b0_d = nc.dram_tensor("b0s", [128, OT], F32, kind="ExternalInput")
    g_d = [nc.dram_tensor(f"g{k}s", [128, OT], F32, kind="ExternalInput") for k in (1, 2, 3)]
    be_d = [nc.dram_tensor(f"be{k}s", [128, OT], F32, kind="ExternalInput") for k in (1, 2, 3)]
    bo_d = nc.dram_tensor("bo", [1, 1], F32, kind="ExternalInput")
    out_d = nc.dram_tensor("out", [rows, 1], F32, kind="ExternalOutput")

    ident_d = nc.dram_tensor("ident", [128, 128], F32R, kind="ExternalInput")
    ones_d = nc.dram_tensor("onesm", [128, 128], F32R, kind="ExternalInput")

    with tile.TileContext(nc) as tc, ExitStack() as ctx:
        const = ctx.enter_context(tc.tile_pool(name="const", bufs=1))
        p_xin = ctx.enter_context(tc.tile_pool(name="xin", bufs=4))
        p_xt = ctx.enter_context(tc.tile_pool(name="xt", bufs=2))
        p_h = ctx.enter_context(tc.tile_pool(name="h", bufs=2))
        p_zc = ctx.enter_context(tc.tile_pool(name="zc", bufs=8))
        p_zsq = ctx.enter_context(tc.tile_pool(name="zsq", bufs=2))
        p_u = ctx.enter_context(tc.tile_pool(name="u", bufs=2))
        p_sq = ctx.enter_context(tc.tile_pool(name="sq", bufs=2))
        p_rc = ctx.enter_context(tc.tile_pool(name="rc", bufs=2))
        p_ob = ctx.enter_context(tc.tile_pool(name="ob", bufs=2))
        ps_z = ctx.enter_context(tc.tile_pool(name="psz", bufs=5, space="PSUM"))
        ps_tp = ctx.enter_context(tc.tile_pool(name="pstp", bufs=2, space="PSUM"))
        ps_st = ctx.enter_context(tc.tile_pool(name="psst", bufs=1, space="PSUM"))

        # --- resident constants ---
        w0t = const.tile([128, KT0 * OT * 128], F32R)
        nc.sync.dma_start(w0t[:], w0_d[:])
        wht = []
        for k in range(3):
            t = const.tile([128, KT * OT * 128], F32R, tag=f"w{k + 1}t")
            nc.sync.dma_start(t[:], wh_d[k][:])
            wht.append(t)
        wot = const.tile([128, KT], F32R)
        nc.sync.dma_start(wot[:], wo_d[:])
        b0t = const.tile([128, OT], F32)
        nc.sync.dma_start(b0t[:], b0_d[:])
        gt, bet = [], []
        for k in range(3):
            g = const.tile([128, OT], F32, tag=f"g{k + 1}t")
            nc.sync.dma_start(g[:], g_d[k][:])
            gt.append(g)
            b = const.tile([128, OT], F32, tag=f"be{k + 1}t")
            nc.sync.dma_start(b[:], be_d[k][:])
            bet.append(b)
        bot = const.tile([1, 1], F32)
        nc.sync.dma_start(bot[:], bo_d[:])
        idt = const.tile([128, 128], F32R)
        nc.sync.dma_start(idt[:], ident_d[:])
        onest = const.tile([128, 128], F32R)
        nc.sync.dma_start(onest[:], ones_d[:])
        epst = const.tile([128, 1], F32)
        nc.vector.memset(epst[:], LN_EPS)

        w0v = w0t[:].rearrange("p (kt ot m) -> p kt ot m", kt=KT0, ot=OT)
        whv = [t[:].rearrange("p (kt ot m) -> p kt ot m", kt=KT, ot=OT) for t in wht]

        for c in range(nchunk):
            # ---- load X chunk and transpose to feature-major xT[feat, rows]
            xin = []
            for rg in range(4):
                t = p_xin.tile([128, D_IN], F32R, tag="xin")
                nc.sync.dma_start(t[:], x_d[c * R + rg * 128 : c * R + (rg + 1) * 128, :])
                xin.append(t)
            xt = p_xt.tile([128, KT0, R], F32R, tag="xt")
            for rg in range(4):
                for ft in range(KT0):
                    tp = ps_tp.tile([128, 128], F32, tag="tp")
                    nc.tensor.transpose(
                        _r(tp[:]), _r(xin[rg][:, ft * 128 : (ft + 1) * 128]), _r(idt[:])
                    )
                    nc.scalar.copy(xt[:, ft, rg * 128 : (rg + 1) * 128], tp[:])

            # ---- layer 0: h0 = relu(X @ W0 + b0)
            hprev = p_h.tile([128, KT, R], F32R, tag="h")
            for ot in range(OT):
                z = ps_z.tile([128, R], F32, tag="z")
                for kt in range(KT0):
                    nc.tensor.matmul(
                        z[:],
                        _r(w0v[:, kt, ot, :]),
                        _r(xt[:, kt, :]),
                        start=(kt == 0),
                        stop=(kt == KT0 - 1),
                    )
                nc.scalar.activation(
                    hprev[:, ot, :], z[:], AF.Relu, bias=b0t[:, ot : ot + 1], scale=1.0
                )

            # ---- hidden layers 1..3: h = relu(LN(h @ Wc) * g + be)
            for k in range(3):
                hn = p_h.tile([128, KT, R], F32R, tag="h")
                zcs = []
                zsqs = []
                for ot in range(OT):
                    z = ps_z.tile([128, R], F32, tag="z")
                    for kt in range(KT):
                        nc.tensor.matmul(
                            z[:],
                            _r(whv[k][:, kt, ot, :]),
                            _r(hprev[:, kt, :]),
                            start=(kt == 0),
                            stop=(kt == KT - 1),
                        )
                    zc = p_zc.tile([128, R], F32, tag="zc")
                    nc.scalar.copy(zc[:], z[:])
                    zsq = p_zsq.tile([128, R], F32R, tag="zsq")
                    nc.vector.tensor_mul(zsq[:], zc[:], zc[:])
                    zcs.append(zc)
                    zsqs.append(zsq)
                st = ps_st.tile([128, R], F32, tag="st")
                for ot in range(OT):
                    nc.tensor.matmul(
                        st[:],
                        _r(onest[:]),
                        _r(zsqs[ot][:]),
                        start=(ot == 0),
                        stop=(ot == OT - 1),
                        skip_group_check=True,
                    )
                # sqrt(var + eps), var = sumsq / 1024 ; then 1/sqrt on DVE
                sq = p_sq.tile([128, R], F32, tag="sq")
                nc.scalar.activation(
                    sq[:], st[:], AF.Sqrt, bias=epst[:], scale=1.0 / W_HID
                )
                rc = p_rc.tile([128, R], F32, tag="rc")
                nc.vector.reciprocal(rc[:], sq[:])
                for ot in range(OT):
                    u = p_u.tile([128, R], F32, tag="u")
                    nc.vector.scalar_tensor_tensor(
                        u[:], zcs[ot][:], gt[k][:, ot : ot + 1], rc[:],
                        op0=ALU.mult, op1=ALU.mult,
                    )
                    nc.scalar.activation(
                        hn[:, ot, :], u[:], AF.Relu, bias=bet[k][:, ot : ot + 1],
                        scale=1.0,
                    )
                hprev = hn

            # ---- output layer: y = relu(h3 @ Wout + bout)
            zo = ps_st.tile([128, R], F32, tag="st")
            for kt in range(KT):
                nc.tensor.matmul(
                    zo[:1, :],
                    _r(wot[:, kt : kt + 1]),
                    _r(hprev[:, kt, :]),
                    start=(kt == 0),
                    stop=(kt == KT - 1),
                )
            ob = p_ob.tile([1, R], F32, tag="ob")
            nc.scalar.activation(ob[:], zo[:1, :], AF.Relu, bias=bot[:, :], scale=1.0)
            nc.sync.dma_start(out_d[c * R : (c + 1) * R, :], ob[:])

    nc.finalize()
    return nc


def build_nc_fast(rows=ROWS):
    """Fast variant, valid when be1..be3 == 0 and g1..g3 > 0 elementwise.

    Uses fp16 matmul operands (1 cycle/row on the PE vs ~2 for fp32r) and
    defers the LayerNorm scaling: LN is invariant to positive per-row scaling
    of its input, and relu commutes with positive per-row scales, so each
    hidden layer just passes h~ = relu(z_c * g) forward unnormalized. The
    cumulative squared scale follows d2_k = m~_k + eps * d2_{k-1} (m~_k =
    weighted mean of z~^2 via a (1/(1024 g^2))-stationary matmul), and a
    single rsqrt per chunk rescales the output-layer logits.
    """
    nchunk = rows // R
    F16 = mybir.dt.float16
    nc = bacc.Bacc()

    x_d = nc.dram_tensor("x", [rows, D_IN], F16, kind="ExternalInput")
    w0_d = nc.dram_tensor("w0s", [128, KT0 * OT * 128], F16, kind="ExternalInput")
    wh_d = [
        nc.dram_tensor(f"w{k}s", [128, KT * OT * 128], F16, kind="ExternalInput")
        for k in (1, 2, 3)
    ]
    sw_d = [
        nc.dram_tensor(f"sw{k}s", [128, OT * 128], F16, kind="ExternalInput")
        for k in (1, 2, 3)
    ]
    wo_d = nc.dram_tensor("wos", [128, KT], F16, kind="ExternalInput")
    b0_d = nc.dram_tensor("b0s", [128, OT], F32, kind="ExternalInput")
    bo_d = nc.dram_tensor("bo", [1, 1], F32, kind="ExternalInput")
    out_d = nc.dram_tensor("out", [rows, 1], F32, kind="ExternalOutput")

    with tile.TileContext(nc) as tc, ExitStack() as ctx:
        const = ctx.enter_context(tc.tile_pool(name="const", bufs=1))
        p_xt = ctx.enter_context(tc.tile_pool(name="xt", bufs=3))
        p_h = ctx.enter_context(tc.tile_pool(name="h", bufs=3))
        p_zsq = ctx.enter_context(tc.tile_pool(name="zsq", bufs=3))
        p_d2 = ctx.enter_context(tc.tile_pool(name="d2", bufs=4))
        p_s = ctx.enter_context(tc.tile_pool(name="s", bufs=2))
        p_ob = ctx.enter_context(tc.tile_pool(name="ob", bufs=3))
        ps_z = ctx.enter_context(tc.tile_pool(name="psz", bufs=4, space="PSUM"))
        ps_st = ctx.enter_context(tc.tile_pool(name="psst", bufs=3, space="PSUM"))
        ps_zo = ctx.enter_context(tc.tile_pool(name="pszo", bufs=1, space="PSUM"))

        # --- resident constants ---
        w0t = const.tile([128, KT0 * OT * 128], F16)
        nc.sync.dma_start(w0t[:], w0_d[:])
        wht = []
        swt = []
        for k in range(3):
            t = const.tile([128, KT * OT * 128], F16, tag=f"w{k + 1}t")
            nc.sync.dma_start(t[:], wh_d[k][:])
            wht.append(t)
            t = const.tile([128, OT, 128], F16, tag=f"sw{k + 1}t")
            nc.sync.dma_start(t[:], sw_d[k][:])
            swt.append(t)
        wot = const.tile([128, KT], F16)
        nc.sync.dma_start(wot[:], wo_d[:])
        b0t = const.tile([128, OT], F32)
        nc.sync.dma_start(b0t[:], b0_d[:])
        bot = const.tile([1, 1], F32)
        nc.sync.dma_start(bot[:], bo_d[:])
        epst = const.tile([128, 1], F32)
        nc.vector.memset(epst[:], LN_EPS)
        zerot = const.tile([128, 1], F32)
        nc.vector.memset(zerot[:], 0.0)

        w0v = w0t[:].rearrange("p (kt ot m) -> p kt ot m", kt=KT0, ot=OT)
        whv = [t[:].rearrange("p (kt ot m) -> p kt ot m", kt=KT, ot=OT) for t in wht]

        pend = None  # delayed stats matmul: (st, sw_tile, ot, zsq)

        def flush_stats(nc):
            nonlocal pend
            if pend is not None:
                st_, sw_, ot_, zsq_ = pend
                nc.tensor.matmul(
                    st_[:], sw_[:, ot_, :], zsq_[:],
                    start=(ot_ == 0), stop=(ot_ == OT - 1),
                    skip_group_check=True,
                )
                pend = None

        for c in range(nchunk):
            # ---- X chunk straight to feature-major xT[feat, rows] via xbar DMA
            xt = p_xt.tile([128, KT0, R], F16, tag="xt")
            for ft in range(KT0):
                nc.sync.dma_start(
                    xt[:, ft, :],
                    x_d[c * R : (c + 1) * R, ft * 128 : (ft + 1) * 128],
                    transpose=True,
                )

            # ---- layer 0: h0 = relu(X @ W0 + b0)   (relu on DVE)
            hprev = p_h.tile([128, KT, R], F16, tag="h")
            for ot in range(OT):
                z = ps_z.tile([128, R], F32, tag="z")
                for kt in range(KT0):
                    nc.tensor.matmul(
                        z[:], w0v[:, kt, ot, :], xt[:, kt, :],
                        start=(kt == 0), stop=(kt == KT0 - 1),
                    )
                flush_stats(nc)
                nc.vector.tensor_scalar(
                    hprev[:, ot, :], z[:], b0t[:, ot : ot + 1], 0.0,
                    op0=ALU.add, op1=ALU.max,
                )

            # ---- hidden layers: h~ = relu(h~prev @ (Wc*g)); m~ accumulated on PE
            # stats matmuls are emitted one output-tile late so they never make
            # the PE (strict FIFO) wait on the ACT square of the current tile.
            sts = []
            for k in range(3):
                hn = p_h.tile([128, KT, R], F16, tag="h")
                st = ps_st.tile([128, R], F32, tag="st")
                for ot in range(OT):
                    z = ps_z.tile([128, R], F32, tag="z")
                    for kt in range(KT):
                        nc.tensor.matmul(
                            z[:], whv[k][:, kt, ot, :], hprev[:, kt, :],
                            start=(kt == 0), stop=(kt == KT - 1),
                        )
                    flush_stats(nc)
                    zsq = p_zsq.tile([128, R], F16, tag="zsq")
                    nc.scalar.activation(
                        zsq[:], z[:], AF.Square, bias=zerot[:], scale=1.0
                    )
                    nc.vector.tensor_scalar(
                        hn[:, ot, :], z[:], 0.0, None, op0=ALU.max
                    )
                    pend = (st, swt[k], ot, zsq)
                sts.append(st)
                hprev = hn

            # ---- output layer matmuls (flush last stats after the first ones)
            zo = ps_zo.tile([128, R], F32, tag="zo")
            for kt in range(KT):
                nc.tensor.matmul(
                    zo[:1, :], wot[:, kt : kt + 1], hprev[:, kt, :],
                    start=(kt == 0), stop=(kt == KT - 1),
                )
                if kt == 0:
                    flush_stats(nc)

            # ---- d2 recursion off the three st tiles, then rescale + relu
            d2 = None
            for k in range(3):
                d2n = p_d2.tile([128, R], F32, tag="d2")
                if d2 is None:
                    nc.scalar.activation(
                        d2n[:], sts[k][:], AF.Identity, bias=epst[:], scale=1.0
                    )
                else:
                    nc.vector.scalar_tensor_tensor(
                        d2n[:], d2[:], LN_EPS, sts[k][:], op0=ALU.mult, op1=ALU.add
                    )
                d2 = d2n
            s = p_s.tile([128, R], F32, tag="s")
            nc.scalar.activation(s[:], d2[:], AF.Sqrt, bias=zerot[:], scale=1.0)
            rs = p_s.tile([128, R], F32, tag="rs")
            nc.vector.reciprocal(rs[:1, :], s[:1, :])
            yv = p_ob.tile([1, R], F32, tag="yv")
            nc.vector.tensor_mul(yv[:], zo[:1, :], rs[:1, :])
            ob = p_ob.tile([1, R], F32, tag="ob")
            nc.vector.tensor_scalar(
                ob[:], yv[:], bot[:, :], 0.0, op0=ALU.add, op1=ALU.max
            )
            nc.sync.dma_start(out_d[c * R : (c + 1) * R, :], ob[:])

    nc.finalize()
    return nc


def _prep_inputs(inputs):
    """Host-side weight repack (float64 intermediates for the centering)."""
    f32 = np.float32

    def center(w):
        w64 = np.asarray(w, np.float64)
        return (w64 - w64.mean(axis=1, keepdims=True)).astype(f32)

    def pack_w(w, kt):
        # [kt*128, 1024] -> sbuf layout [p, kt, ot, m] flattened
        return (
            np.ascontiguousarray(
                np.asarray(w, f32).reshape(kt, 128, OT, 128).transpose(1, 0, 2, 3)
            ).reshape(128, kt * OT * 128)
        )

    def pack_v(v):
        # [1024] -> [128, OT] with [:, ot] = v[ot*128:(ot+1)*128]
        return np.ascontiguousarray(np.asarray(v, f32).reshape(OT, 128).T)

    common = {
        "w0s": pack_w(inputs["W0"], KT0),
        "w1s": pack_w(center(inputs["W1"]), KT),
        "w2s": pack_w(center(inputs["W2"]), KT),
        "w3s": pack_w(center(inputs["W3"]), KT),
        "wos": np.ascontiguousarray(np.asarray(inputs["Wout"], f32).reshape(KT, 128).T),
        "b0s": pack_v(inputs["b0"]),
        "g1s": pack_v(inputs["g1"]),
        "be1s": pack_v(inputs["be1"]),
        "g2s": pack_v(inputs["g2"]),
        "be2s": pack_v(inputs["be2"]),
        "g3s": pack_v(inputs["g3"]),
        "be3s": pack_v(inputs["be3"]),
        "bo": np.asarray(inputs["bout"], f32).reshape(1, 1),
        "ident": np.eye(128, dtype=f32),
        "onesm": np.ones((128, 128), dtype=f32),
    }
    return common


def _prep_inputs_fast(inputs):
    """Host prep for the fast (be==0, g>0) variant: fp16 weights, g folded
    into the hidden weights, 1/(1024 g^2) folded into the stats stationary."""
    f16 = np.float16

    def center(w):
        w64 = np.asarray(w, np.float64)
        return w64 - w64.mean(axis=1, keepdims=True)

    def pack_w(w64, kt):
        return np.ascontiguousarray(
            w64.reshape(kt, 128, OT, 128).transpose(1, 0, 2, 3)
        ).reshape(128, kt * OT * 128).astype(f16)

    def stats_w(g):
        g64 = np.asarray(g, np.float64)
        vals = (1.0 / (W_HID * g64 * g64)).reshape(OT, 128)  # [ot, p]
        return np.ascontiguousarray(
            np.broadcast_to(vals.T[:, :, None], (128, OT, 128))
        ).reshape(128, OT * 128).astype(f16)

    common = {
        "w0s": pack_w(np.asarray(inputs["W0"], np.float64), KT0),
        "wos": np.ascontiguousarray(
            np.asarray(inputs["Wout"], np.float64).reshape(KT, 128).T
        ).astype(f16),
        "b0s": np.ascontiguousarray(
            np.asarray(inputs["b0"], np.float32).reshape(OT, 128).T
        ),
        "bo": np.asarray(inputs["bout"], np.float32).reshape(1, 1),
    }
    for k in (1, 2, 3):
        g64 = np.asarray(inputs[f"g{k}"], np.float64)
        common[f"w{k}s"] = pack_w(center(inputs[f"W{k}"]) * g64[None, :], KT)
        common[f"sw{k}s"] = stats_w(g64)
    return common


_NC_CACHE = {}


def _get_nc(fast):
    key = "fast" if fast else "general"
    if key not in _NC_CACHE:
        _NC_CACHE[key] = build_nc_fast() if fast else build_nc()
    return _NC_CACHE[key]


def _is_fast_ok(inputs):
    return all(
        np.all(np.asarray(inputs[f"be{k}"]) == 0)
        and np.all(np.asarray(inputs[f"g{k}"]) > 0)
        for k in (1, 2, 3)
    )


def _run(inputs, trace=False):
    fast = _is_fast_ok(inputs)
    common = _prep_inputs_fast(inputs) if fast else _prep_inputs(inputs)
    xdt = np.float16 if fast else np.float32
    x = np.ascontiguousarray(np.asarray(inputs["descriptors"], np.float32).astype(xdt))
    shards = x.reshape(N_CORES, ROWS, D_IN)
    in_maps = [dict(common, x=np.ascontiguousarray(shards[i])) for i in range(N_CORES)]
    nc = _get_nc(fast)
    res = run_bass_kernel_spmd(nc, in_maps, core_ids=list(range(N_CORES)), trace=trace)
    out = np.concatenate([res.results[i]["out"] for i in range(N_CORES)], axis=0)
    return out.astype(np.float32), res


def kernel(**inputs):
    out, _ = _run(inputs, trace=False)
    return out


def kernel_traced(**inputs):
    out, res = _run(inputs, trace=True)
    return out, res


# revision 15
# speedup vs baseline: 1.5601x; 1.0082x over previous
"""Trainium2 Bass kernel for nn_CoreRelu_83863531422003 (5-layer MLP).

Network (per reference):
    h0 = relu(X @ W0 + b0)                      X:[N,512] W0:[512,1024]
    hk = relu(LN(h_{k-1} @ Wk) * gk + bek)      Wk:[1024,1024], k=1..3
    y  = relu(h3 @ Wout + bout)                 Wout:[1024,1]

Sharding: data-parallel over rows across 8 NeuronCores (8192 rows/core),
weights replicated. No communication.

On-chip layout ("option B", feature-major): activations live transposed in
SBUF as [feat(partition), rows(free)]; weights are the stationary matmul
operand so layer outputs stay feature-major and no per-layer transposes are
needed. Only X is transposed once (PE transpose) on entry.

LayerNorm: host pre-centers hidden weights (W_c = W - W.mean(axis=1)) so
z = h @ W_c is exactly mean-free; variance = sumsq(z)/1024, computed with a
ones[128x128]-stationary matmul over z^2 which also replicates the per-row
sum across all 128 partitions (needed for the feature-major apply).

Matmuls run as float32r (1 cycle/row for moving free dim >=256 vs 4 for
plain fp32).
"""

import numpy as np
from contextlib import ExitStack

import concourse.bass as bass
import concourse.bacc as bacc
import concourse.tile as tile
from concourse import mybir
from concourse.bass_utils import run_bass_kernel_spmd

N_CORES = 8
N_FULL = 65536
D_IN = 512
W_HID = 1024
ROWS = N_FULL // N_CORES      # 8192 rows per core
R = 512                       # rows per chunk (one PSUM bank of fp32)
NCHUNK = ROWS // R            # 16
KT0 = D_IN // 128             # 4 k-tiles for layer 0
KT = W_HID // 128             # 8 k-tiles for hidden layers
OT = W_HID // 128             # 8 output-feature tiles
LN_EPS = 1e-6

F32 = mybir.dt.float32
F32R = mybir.dt.float32r
MM_DT = F32R     # flip to F32 if fp32r numerics prove too loose

AF = mybir.ActivationFunctionType
ALU = mybir.AluOpType


def _r(ap):
    """Bitcast an AP to the matmul dtype."""
    if MM_DT == F32:
        return ap
    return ap.bitcast(MM_DT)


def build_nc(rows=ROWS):
    nchunk = rows // R
    nc = bacc.Bacc()

    x_d = nc.dram_tensor("x", [rows, D_IN], F32R, kind="ExternalInput")
    w0_d = nc.dram_tensor("w0s", [128, KT0 * OT * 128], F32R, kind="ExternalInput")
    wh_d = [
        nc.dram_tensor(f"w{k}s", [128, KT * OT * 128], F32R, kind="ExternalInput")
        for k in (1, 2, 3)
    ]
    wo_d = nc.dram_tensor("wos", [128, KT], F32R, kind="ExternalInput")
    b0_d = nc.dram_tensor("b0s", [128, OT], F32, kind="ExternalInput")
    g_d = [nc.dram_tensor(f"g{k}s", [128, OT], F32, kind="ExternalInput") for k in (1, 2, 3)]
    be_d = [nc.dram_tensor(f"be{k}s", [128, OT], F32, kind="ExternalInput") for k in (1, 2, 3)]
    bo_d = nc.dram_tensor("bo", [1, 1], F32, kind="ExternalInput")
    out_d = nc.dram_tensor("out", [rows, 1], F32, kind="ExternalOutput")

    ident_d = nc.dram_tensor("ident", [128, 128], F32R, kind="ExternalInput")
    ones_d = nc.dram_tensor("onesm", [128, 128], F32R, kind="ExternalInput")

    with tile.TileContext(nc) as tc, ExitStack() as ctx:
        const = ctx.enter_context(tc.tile_pool(name="const", bufs=1))
        p_xin = ctx.enter_context(tc.tile_pool(name="xin", bufs=4))
        p_xt = ctx.enter_context(tc.tile_pool(name="xt", bufs=2))
        p_h = ctx.enter_context(tc.tile_pool(name="h", bufs=2))
        p_zc = ctx.enter_context(tc.tile_pool(name="zc", bufs=8))
        p_zsq = ctx.enter_context(tc.tile_pool(name="zsq", bufs=2))
        p_u = ctx.enter_context(tc.tile_pool(name="u", bufs=2))
        p_sq = ctx.enter_context(tc.tile_pool(name="sq", bufs=2))
        p_rc = ctx.enter_context(tc.tile_pool(name="rc", bufs=2))
        p_ob = ctx.enter_context(tc.tile_pool(name="ob", bufs=2))
        ps_z = ctx.enter_context(tc.tile_pool(name="psz", bufs=5, space="PSUM"))
        ps_tp = ctx.enter_context(tc.tile_pool(name="pstp", bufs=2, space="PSUM"))
        ps_st = ctx.enter_context(tc.tile_pool(name="psst", bufs=1, space="PSUM"))

        # --- resident constants ---
        w0t = const.tile([128, KT0 * OT * 128], F32R)
        nc.sync.dma_start(w0t[:], w0_d[:])
        wht = []
        for k in range(3):
            t = const.tile([128, KT * OT * 128], F32R, tag=f"w{k + 1}t")
            nc.sync.dma_start(t[:], wh_d[k][:])
            wht.append(t)
        wot = const.tile([128, KT], F32R)
        nc.sync.dma_start(wot[:], wo_d[:])
        b0t = const.tile([128, OT], F32)
        nc.sync.dma_start(b0t[:], b0_d[:])
        gt, bet = [], []
        for k in range(3):
            g = const.tile([128, OT], F32, tag=f"g{k + 1}t")
            nc.sync.dma_start(g[:], g_d[k][:])
            gt.append(g)
            b = const.tile([128, OT], F32, tag=f"be{k + 1}t")
            nc.sync.dma_start(b[:], be_d[k][:])
            bet.append(b)
        bot = const.tile([1, 1], F32)
        nc.sync.dma_start(bot[:], bo_d[:])
        idt = const.tile([128, 128], F32R)
        nc.sync.dma_start(idt[:], ident_d[:])
        onest = const.tile([128, 128], F32R)
        nc.sync.dma_start(onest[:], ones_d[:])
        epst = const.tile([128, 1], F32)
        nc.vector.memset(epst[:], LN_EPS)

        w0v = w0t[:].rearrange("p (kt ot m) -> p kt ot m", kt=KT0, ot=OT)
        whv = [t[:].rearrange("p (kt ot m) -> p kt ot m", kt=KT, ot=OT) for t in wht]

        for c in range(nchunk):
            # ---- load X chunk and transpose to feature-major xT[feat, rows]
            xin = []
            for rg in range(4):
                t = p_xin.tile([128, D_IN], F32R, tag="xin")
                nc.sync.dma_start(t[:], x_d[c * R + rg * 128 : c * R + (rg + 1) * 128, :])
                xin.append(t)
            xt = p_xt.tile([128, KT0, R], F32R, tag="xt")
            for rg in range(4):
                for ft in range(KT0):
                    tp = ps_tp.tile([128, 128], F32, tag="tp")
                    nc.tensor.transpose(
                        _r(tp[:]), _r(xin[rg][:, ft * 128 : (ft + 1) * 128]), _r(idt[:])
                    )
                    nc.scalar.copy(xt[:, ft, rg * 128 : (rg + 1) * 128], tp[:])

            # ---- layer 0: h0 = relu(X @ W0 + b0)
            hprev = p_h.tile([128, KT, R], F32R, tag="h")
            for ot in range(OT):
                z = ps_z.tile([128, R], F32, tag="z")
                for kt in range(KT0):
                    nc.tensor.matmul(
                        z[:],
                        _r(w0v[:, kt, ot, :]),
                        _r(xt[:, kt, :]),
                        start=(kt == 0),
                        stop=(kt == KT0 - 1),
                    )
                nc.scalar.activation(
                    hprev[:, ot, :], z[:], AF.Relu, bias=b0t[:, ot : ot + 1], scale=1.0
                )

            # ---- hidden layers 1..3: h = relu(LN(h @ Wc) * g + be)
            for k in range(3):
                hn = p_h.tile([128, KT, R], F32R, tag="h")
                zcs = []
                zsqs = []
                for ot in range(OT):
                    z = ps_z.tile([128, R], F32, tag="z")
                    for kt in range(KT):
                        nc.tensor.matmul(
                            z[:],
                            _r(whv[k][:, kt, ot, :]),
                            _r(hprev[:, kt, :]),
                            start=(kt == 0),
                            stop=(kt == KT - 1),
                        )
                    zc = p_zc.tile([128, R], F32, tag="zc")
                    nc.scalar.copy(zc[:], z[:])
                    zsq = p_zsq.tile([128, R], F32R, tag="zsq")
                    nc.vector.tensor_mul(zsq[:], zc[:], zc[:])
                    zcs.append(zc)
                    zsqs.append(zsq)
                st = ps_st.tile([128, R], F32, tag="st")
                for ot in range(OT):
                    nc.tensor.matmul(
                        st[:],
                        _r(onest[:]),
                        _r(zsqs[ot][:]),
                        start=(ot == 0),
                        stop=(ot == OT - 1),
                        skip_group_check=True,
                    )
                # sqrt(var + eps), var = sumsq / 1024 ; then 1/sqrt on DVE
                sq = p_sq.tile([128, R], F32, tag="sq")
                nc.scalar.activation(
                    sq[:], st[:], AF.Sqrt, bias=epst[:], scale=1.0 / W_HID
                )
                rc = p_rc.tile([128, R], F32, tag="rc")
                nc.vector.reciprocal(rc[:], sq[:])
                for ot in range(OT):
                    u = p_u.tile([128, R], F32, tag="u")
                    nc.vector.scalar_tensor_tensor(
                        u[:], zcs[ot][:], gt[k][:, ot : ot + 1], rc[:],
                        op0=ALU.mult, op1=ALU.mult,
                    )
                    nc.scalar.activation(
                        hn[:, ot, :], u[:], AF.Relu, bias=bet[k][:, ot : ot + 1],
                        scale=1.0,
                    )
                hprev = hn

            # ---- output layer: y = relu(h3 @ Wout + bout)
            zo = ps_st.tile([128, R], F32, tag="st")
            for kt in range(KT):
                nc.tensor.matmul(
                    zo[:1, :],
                    _r(wot[:, kt : kt + 1]),
                    _r(hprev[:, kt, :]),
                    start=(kt == 0),
                    stop=(kt == KT - 1),
                )
            ob = p_ob.tile([1, R], F32, tag="ob")
            nc.scalar.activation(ob[:], zo[:1, :], AF.Relu, bias=bot[:, :], scale=1.0)
            nc.sync.dma_start(out_d[c * R : (c + 1) * R, :], ob[:])

    nc.finalize()
    return nc


def build_nc_fast(rows=ROWS):
    """Fast variant, valid when be1..be3 == 0 and g1..g3 > 0 elementwise.

    Uses fp16 matmul operands (1 cycle/row on the PE vs ~2 for fp32r) and
    defers the LayerNorm scaling: LN is invariant to positive per-row scaling
    of its input, and relu commutes with positive per-row scales, so each
    hidden layer just passes h~ = relu(z_c * g) forward unnormalized. The
    cumulative squared scale follows d2_k = m~_k + eps * d2_{k-1} (m~_k =
    weighted mean of z~^2 via a (1/(1024 g^2))-stationary matmul), and a
    single rsqrt per chunk rescales the output-layer logits.
    """
    nchunk = rows // R
    F16 = mybir.dt.float16
    nc = bacc.Bacc()

    x_d = nc.dram_tensor("x", [rows, D_IN], F16, kind="ExternalInput")
    w0_d = nc.dram_tensor("w0s", [128, KT0 * OT * 128], F16, kind="ExternalInput")
    wh_d = [
        nc.dram_tensor(f"w{k}s", [128, KT * OT * 128], F16, kind="ExternalInput")
        for k in (1, 2, 3)
    ]
    sw_d = [
        nc.dram_tensor(f"sw{k}s", [128, OT * 128], F16, kind="ExternalInput")
        for k in (1, 2, 3)
    ]
    wo_d = nc.dram_tensor("wos", [128, KT], F16, kind="ExternalInput")
    b0_d = nc.dram_tensor("b0s", [128, OT], F32, kind="ExternalInput")
    bo_d = nc.dram_tensor("bo", [1, 1], F32, kind="ExternalInput")
    out_d = nc.dram_tensor("out", [rows, 1], F32, kind="ExternalOutput")

    with tile.TileContext(nc) as tc, ExitStack() as ctx:
        const = ctx.enter_context(tc.tile_pool(name="const", bufs=1))
        p_xt = ctx.enter_context(tc.tile_pool(name="xt", bufs=3))
        p_h = ctx.enter_context(tc.tile_pool(name="h", bufs=3))
        p_zsq = ctx.enter_context(tc.tile_pool(name="zsq", bufs=3))
        p_d2 = ctx.enter_context(tc.tile_pool(name="d2", bufs=4))
        p_s = ctx.enter_context(tc.tile_pool(name="s", bufs=2))
        p_ob = ctx.enter_context(tc.tile_pool(name="ob", bufs=3))
        ps_z = ctx.enter_context(tc.tile_pool(name="psz", bufs=4, space="PSUM"))
        ps_st = ctx.enter_context(tc.tile_pool(name="psst", bufs=3, space="PSUM"))
        ps_zo = ctx.enter_context(tc.tile_pool(name="pszo", bufs=1, space="PSUM"))

        # --- resident constants ---
        w0t = const.tile([128, KT0 * OT * 128], F16)
        nc.gpsimd.dma_start(w0t[:], w0_d[:])
        wht = []
        swt = []
        for k in range(3):
            t = const.tile([128, KT * OT * 128], F16, tag=f"w{k + 1}t")
            nc.gpsimd.dma_start(t[:], wh_d[k][:])
            wht.append(t)
            t = const.tile([128, OT, 128], F16, tag=f"sw{k + 1}t")
            nc.gpsimd.dma_start(t[:], sw_d[k][:])
            swt.append(t)
        wot = const.tile([128, KT], F16)
        nc.gpsimd.dma_start(wot[:], wo_d[:])
        b0t = const.tile([128, OT], F32)
        nc.gpsimd.dma_start(b0t[:], b0_d[:])
        bot = const.tile([1, 1], F32)
        nc.gpsimd.dma_start(bot[:], bo_d[:])
        epst = const.tile([128, 1], F32)
        nc.vector.memset(epst[:], LN_EPS)
        zerot = const.tile([128, 1], F32)
        nc.vector.memset(zerot[:], 0.0)

        w0v = w0t[:].rearrange("p (kt ot m) -> p kt ot m", kt=KT0, ot=OT)
        whv = [t[:].rearrange("p (kt ot m) -> p kt ot m", kt=KT, ot=OT) for t in wht]

        pend = None  # delayed stats matmul: (st, sw_tile, ot, zsq)

        def flush_stats(nc):
            nonlocal pend
            if pend is not None:
                st_, sw_, ot_, zsq_ = pend
                nc.tensor.matmul(
                    st_[:], sw_[:, ot_, :], zsq_[:],
                    start=(ot_ == 0), stop=(ot_ == OT - 1),
                    skip_group_check=True,
                )
                pend = None

        for c in range(nchunk):
            # ---- X chunk straight to feature-major xT[feat, rows] via xbar DMA
            xt = p_xt.tile([128, KT0, R], F16, tag="xt")
            for ft in range(KT0):
                nc.sync.dma_start(
                    xt[:, ft, :],
                    x_d[c * R : (c + 1) * R, ft * 128 : (ft + 1) * 128],
                    transpose=True,
                )

            # ---- layer 0: h0 = relu(X @ W0 + b0)   (relu on DVE)
            hprev = p_h.tile([128, KT, R], F16, tag="h")
            for ot in range(OT):
                z = ps_z.tile([128, R], F32, tag="z")
                for kt in range(KT0):
                    nc.tensor.matmul(
                        z[:], w0v[:, kt, ot, :], xt[:, kt, :],
                        start=(kt == 0), stop=(kt == KT0 - 1),
                    )
                flush_stats(nc)
                nc.scalar.activation(
                    hprev[:, ot, :], z[:], AF.Relu, bias=b0t[:, ot : ot + 1],
                    scale=1.0,
                )

            # ---- hidden layers: h~ = relu(h~prev @ (Wc*g)); m~ accumulated on PE
            # stats matmuls are emitted one output-tile late so they never make
            # the PE (strict FIFO) wait on the ACT square of the current tile.
            sts = []
            for k in range(3):
                hn = p_h.tile([128, KT, R], F16, tag="h")
                st = ps_st.tile([128, R], F32, tag="st")
                for ot in range(OT):
                    z = ps_z.tile([128, R], F32, tag="z")
                    for kt in range(KT):
                        nc.tensor.matmul(
                            z[:], whv[k][:, kt, ot, :], hprev[:, kt, :],
                            start=(kt == 0), stop=(kt == KT - 1),
                        )
                    flush_stats(nc)
                    zsq = p_zsq.tile([128, R], F16, tag="zsq")
                    nc.scalar.activation(
                        zsq[:], z[:], AF.Square, bias=zerot[:], scale=1.0
                    )
                    nc.scalar.activation(
                        hn[:, ot, :], z[:], AF.Relu, bias=zerot[:], scale=1.0
                    )
                    pend = (st, swt[k], ot, zsq)
                sts.append(st)
                hprev = hn

            # ---- output layer matmuls (flush last stats after the first ones)
            zo = ps_zo.tile([128, R], F32, tag="zo")
            for kt in range(KT):
                nc.tensor.matmul(
                    zo[:1, :], wot[:, kt : kt + 1], hprev[:, kt, :],
                    start=(kt == 0), stop=(kt == KT - 1),
                )
                if kt == 0:
                    flush_stats(nc)

            # ---- d2 recursion off the three st tiles, then rescale + relu
            d2 = None
            for k in range(3):
                d2n = p_d2.tile([128, R], F32, tag="d2")
                if d2 is None:
                    nc.scalar.activation(
                        d2n[:], sts[k][:], AF.Identity, bias=epst[:], scale=1.0
                    )
                else:
                    nc.vector.scalar_tensor_tensor(
                        d2n[:], d2[:], LN_EPS, sts[k][:], op0=ALU.mult, op1=ALU.add
                    )
                d2 = d2n
            s = p_s.tile([128, R], F32, tag="s")
            nc.scalar.activation(s[:], d2[:], AF.Sqrt, bias=zerot[:], scale=1.0)
            rs = p_s.tile([128, R], F32, tag="rs")
            nc.vector.reciprocal(rs[:1, :], s[:1, :])
            yv = p_ob.tile([1, R], F32, tag="yv")
            nc.vector.tensor_mul(yv[:], zo[:1, :], rs[:1, :])
            ob = p_ob.tile([1, R], F32, tag="ob")
            nc.vector.tensor_scalar(
                ob[:], yv[:], bot[:, :], 0.0, op0=ALU.add, op1=ALU.max
            )
            nc.sync.dma_start(out_d[c * R : (c + 1) * R, :], ob[:])

    nc.finalize()
    return nc


def _prep_inputs(inputs):
    """Host-side weight repack (float64 intermediates for the centering)."""
    f32 = np.float32

    def center(w):
        w64 = np.asarray(w, np.float64)
        return (w64 - w64.mean(axis=1, keepdims=True)).astype(f32)

    def pack_w(w, kt):
        # [kt*128, 1024] -> sbuf layout [p, kt, ot, m] flattened
        return (
            np.ascontiguousarray(
                np.asarray(w, f32).reshape(kt, 128, OT, 128).transpose(1, 0, 2, 3)
            ).reshape(128, kt * OT * 128)
        )

    def pack_v(v):
        # [1024] -> [128, OT] with [:, ot] = v[ot*128:(ot+1)*128]
        return np.ascontiguousarray(np.asarray(v, f32).reshape(OT, 128).T)

    common = {
        "w0s": pack_w(inputs["W0"], KT0),
        "w1s": pack_w(center(inputs["W1"]), KT),
        "w2s": pack_w(center(inputs["W2"]), KT),
        "w3s": pack_w(center(inputs["W3"]), KT),
        "wos": np.ascontiguousarray(np.asarray(inputs["Wout"], f32).reshape(KT, 128).T),
        "b0s": pack_v(inputs["b0"]),
        "g1s": pack_v(inputs["g1"]),
        "be1s": pack_v(inputs["be1"]),
        "g2s": pack_v(inputs["g2"]),
        "be2s": pack_v(inputs["be2"]),
        "g3s": pack_v(inputs["g3"]),
        "be3s": pack_v(inputs["be3"]),
        "bo": np.asarray(inputs["bout"], f32).reshape(1, 1),
        "ident": np.eye(128, dtype=f32),
        "onesm": np.ones((128, 128), dtype=f32),
    }
    return common


def _prep_inputs_fast(inputs):
    """Host prep for the fast (be==0, g>0) variant: fp16 weights, g folded
    into the hidden weights, 1/(1024 g^2) folded into the stats stationary."""
    f16 = np.float16

    def center(w):
        w64 = np.asarray(w, np.float64)
        return w64 - w64.mean(axis=1, keepdims=True)

    def pack_w(w64, kt):
        return np.ascontiguousarray(
            w64.reshape(kt, 128, OT, 128).transpose(1, 0, 2, 3)
        ).reshape(128, kt * OT * 128).astype(f16)

    def stats_w(g):
        g64 = np.asarray(g, np.float64)
        vals = (1.0 / (W_HID * g64 * g64)).reshape(OT, 128)  # [ot, p]
        return np.ascontiguousarray(
            np.broadcast_to(vals.T[:, :, None], (128, OT, 128))
        ).reshape(128, OT * 128).astype(f16)

    common = {
        "w0s": pack_w(np.asarray(inputs["W0"], np.float64), KT0),
        "wos": np.ascontiguousarray(
            np.asarray(inputs["Wout"], np.float64).reshape(KT, 128).T
        ).astype(f16),
        "b0s": np.ascontiguousarray(
            np.asarray(inputs["b0"], np.float32).reshape(OT, 128).T
        ),
        "bo": np.asarray(inputs["bout"], np.float32).reshape(1, 1),
    }
    for k in (1, 2, 3):
        g64 = np.asarray(inputs[f"g{k}"], np.float64)
        common[f"w{k}s"] = pack_w(center(inputs[f"W{k}"]) * g64[None, :], KT)
        common[f"sw{k}s"] = stats_w(g64)
    return common


_NC_CACHE = {}


def _get_nc(fast):
    key = "fast" if fast else "general"
    if key not in _NC_CACHE:
        _NC_CACHE[key] = build_nc_fast() if fast else build_nc()
    return _NC_CACHE[key]


def _is_fast_ok(inputs):
    return all(
        np.all(np.asarray(inputs[f"be{k}"]) == 0)
        and np.all(np.asarray(inputs[f"g{k}"]) > 0)
        for k in (1, 2, 3)
    )


def _run(inputs, trace=False):
    fast = _is_fast_ok(inputs)
    common = _prep_inputs_fast(inputs) if fast else _prep_inputs(inputs)
    xdt = np.float16 if fast else np.float32
    x = np.ascontiguousarray(np.asarray(inputs["descriptors"], np.float32).astype(xdt))
    shards = x.reshape(N_CORES, ROWS, D_IN)
    in_maps = [dict(common, x=np.ascontiguousarray(shards[i])) for i in range(N_CORES)]
    nc = _get_nc(fast)
    res = run_bass_kernel_spmd(nc, in_maps, core_ids=list(range(N_CORES)), trace=trace)
    out = np.concatenate([res.results[i]["out"] for i in range(N_CORES)], axis=0)
    return out.astype(np.float32), res


def kernel(**inputs):
    out, _ = _run(inputs, trace=False)
    return out


def kernel_traced(**inputs):
    out, res = _run(inputs, trace=True)
    return out, res


# revision 16
# speedup vs baseline: 1.5783x; 1.0117x over previous
"""Trainium2 Bass kernel for nn_CoreRelu_83863531422003 (5-layer MLP).

Network (per reference):
    h0 = relu(X @ W0 + b0)                      X:[N,512] W0:[512,1024]
    hk = relu(LN(h_{k-1} @ Wk) * gk + bek)      Wk:[1024,1024], k=1..3
    y  = relu(h3 @ Wout + bout)                 Wout:[1024,1]

Sharding: data-parallel over rows across 8 NeuronCores (8192 rows/core),
weights replicated. No communication.

On-chip layout ("option B", feature-major): activations live transposed in
SBUF as [feat(partition), rows(free)]; weights are the stationary matmul
operand so layer outputs stay feature-major and no per-layer transposes are
needed. Only X is transposed once (PE transpose) on entry.

LayerNorm: host pre-centers hidden weights (W_c = W - W.mean(axis=1)) so
z = h @ W_c is exactly mean-free; variance = sumsq(z)/1024, computed with a
ones[128x128]-stationary matmul over z^2 which also replicates the per-row
sum across all 128 partitions (needed for the feature-major apply).

Matmuls run as float32r (1 cycle/row for moving free dim >=256 vs 4 for
plain fp32).
"""

import numpy as np
from contextlib import ExitStack

import concourse.bass as bass
import concourse.bacc as bacc
import concourse.tile as tile
from concourse import mybir
from concourse.bass_utils import run_bass_kernel_spmd

N_CORES = 8
N_FULL = 65536
D_IN = 512
W_HID = 1024
ROWS = N_FULL // N_CORES      # 8192 rows per core
R = 512                       # rows per chunk (one PSUM bank of fp32)
NCHUNK = ROWS // R            # 16
KT0 = D_IN // 128             # 4 k-tiles for layer 0
KT = W_HID // 128             # 8 k-tiles for hidden layers
OT = W_HID // 128             # 8 output-feature tiles
LN_EPS = 1e-6

F32 = mybir.dt.float32
F32R = mybir.dt.float32r
MM_DT = F32R     # flip to F32 if fp32r numerics prove too loose

AF = mybir.ActivationFunctionType
ALU = mybir.AluOpType


def _r(ap):
    """Bitcast an AP to the matmul dtype."""
    if MM_DT == F32:
        return ap
    return ap.bitcast(MM_DT)


def build_nc(rows=ROWS):
    nchunk = rows // R
    nc = bacc.Bacc()

    x_d = nc.dram_tensor("x", [rows, D_IN], F32R, kind="ExternalInput")
    w0_d = nc.dram_tensor("w0s", [128, KT0 * OT * 128], F32R, kind="ExternalInput")
    wh_d = [
        nc.dram_tensor(f"w{k}s", [128, KT * OT * 128], F32R, kind="ExternalInput")
        for k in (1, 2, 3)
    ]
    wo_d = nc.dram_tensor("wos", [128, KT], F32R, kind="ExternalInput")
    b0_d = nc.dram_tensor("b0s", [128, OT], F32, kind="ExternalInput")
    g_d = [nc.dram_tensor(f"g{k}s", [128, OT], F32, kind="ExternalInput") for k in (1, 2, 3)]
    be_d = [nc.dram_tensor(f"be{k}s", [128, OT], F32, kind="ExternalInput") for k in (1, 2, 3)]
    bo_d = nc.dram_tensor("bo", [1, 1], F32, kind="ExternalInput")
    out_d = nc.dram_tensor("out", [rows, 1], F32, kind="ExternalOutput")

    ident_d = nc.dram_tensor("ident", [128, 128], F32R, kind="ExternalInput")
    ones_d = nc.dram_tensor("onesm", [128, 128], F32R, kind="ExternalInput")

    with tile.TileContext(nc) as tc, ExitStack() as ctx:
        const = ctx.enter_context(tc.tile_pool(name="const", bufs=1))
        p_xin = ctx.enter_context(tc.tile_pool(name="xin", bufs=4))
        p_xt = ctx.enter_context(tc.tile_pool(name="xt", bufs=2))
        p_h = ctx.enter_context(tc.tile_pool(name="h", bufs=2))
        p_zc = ctx.enter_context(tc.tile_pool(name="zc", bufs=8))
        p_zsq = ctx.enter_context(tc.tile_pool(name="zsq", bufs=2))
        p_u = ctx.enter_context(tc.tile_pool(name="u", bufs=2))
        p_sq = ctx.enter_context(tc.tile_pool(name="sq", bufs=2))
        p_rc = ctx.enter_context(tc.tile_pool(name="rc", bufs=2))
        p_ob = ctx.enter_context(tc.tile_pool(name="ob", bufs=2))
        ps_z = ctx.enter_context(tc.tile_pool(name="psz", bufs=5, space="PSUM"))
        ps_tp = ctx.enter_context(tc.tile_pool(name="pstp", bufs=2, space="PSUM"))
        ps_st = ctx.enter_context(tc.tile_pool(name="psst", bufs=1, space="PSUM"))

        # --- resident constants ---
        w0t = const.tile([128, KT0 * OT * 128], F32R)
        nc.sync.dma_start(w0t[:], w0_d[:])
        wht = []
        for k in range(3):
            t = const.tile([128, KT * OT * 128], F32R, tag=f"w{k + 1}t")
            nc.sync.dma_start(t[:], wh_d[k][:])
            wht.append(t)
        wot = const.tile([128, KT], F32R)
        nc.sync.dma_start(wot[:], wo_d[:])
        b0t = const.tile([128, OT], F32)
        nc.sync.dma_start(b0t[:], b0_d[:])
        gt, bet = [], []
        for k in range(3):
            g = const.tile([128, OT], F32, tag=f"g{k + 1}t")
            nc.sync.dma_start(g[:], g_d[k][:])
            gt.append(g)
            b = const.tile([128, OT], F32, tag=f"be{k + 1}t")
            nc.sync.dma_start(b[:], be_d[k][:])
            bet.append(b)
        bot = const.tile([1, 1], F32)
        nc.sync.dma_start(bot[:], bo_d[:])
        idt = const.tile([128, 128], F32R)
        nc.sync.dma_start(idt[:], ident_d[:])
        onest = const.tile([128, 128], F32R)
        nc.sync.dma_start(onest[:], ones_d[:])
        epst = const.tile([128, 1], F32)
        nc.vector.memset(epst[:], LN_EPS)

        w0v = w0t[:].rearrange("p (kt ot m) -> p kt ot m", kt=KT0, ot=OT)
        whv = [t[:].rearrange("p (kt ot m) -> p kt ot m", kt=KT, ot=OT) for t in wht]

        for c in range(nchunk):
            # ---- load X chunk and transpose to feature-major xT[feat, rows]
            xin = []
            for rg in range(4):
                t = p_xin.tile([128, D_IN], F32R, tag="xin")
                nc.sync.dma_start(t[:], x_d[c * R + rg * 128 : c * R + (rg + 1) * 128, :])
                xin.append(t)
            xt = p_xt.tile([128, KT0, R], F32R, tag="xt")
            for rg in range(4):
                for ft in range(KT0):
                    tp = ps_tp.tile([128, 128], F32, tag="tp")
                    nc.tensor.transpose(
                        _r(tp[:]), _r(xin[rg][:, ft * 128 : (ft + 1) * 128]), _r(idt[:])
                    )
                    nc.scalar.copy(xt[:, ft, rg * 128 : (rg + 1) * 128], tp[:])

            # ---- layer 0: h0 = relu(X @ W0 + b0)
            hprev = p_h.tile([128, KT, R], F32R, tag="h")
            for ot in range(OT):
                z = ps_z.tile([128, R], F32, tag="z")
                for kt in range(KT0):
                    nc.tensor.matmul(
                        z[:],
                        _r(w0v[:, kt, ot, :]),
                        _r(xt[:, kt, :]),
                        start=(kt == 0),
                        stop=(kt == KT0 - 1),
                    )
                nc.scalar.activation(
                    hprev[:, ot, :], z[:], AF.Relu, bias=b0t[:, ot : ot + 1], scale=1.0
                )

            # ---- hidden layers 1..3: h = relu(LN(h @ Wc) * g + be)
            for k in range(3):
                hn = p_h.tile([128, KT, R], F32R, tag="h")
                zcs = []
                zsqs = []
                for ot in range(OT):
                    z = ps_z.tile([128, R], F32, tag="z")
                    for kt in range(KT):
                        nc.tensor.matmul(
                            z[:],
                            _r(whv[k][:, kt, ot, :]),
                            _r(hprev[:, kt, :]),
                            start=(kt == 0),
                            stop=(kt == KT - 1),
                        )
                    zc = p_zc.tile([128, R], F32, tag="zc")
                    nc.scalar.copy(zc[:], z[:])
                    zsq = p_zsq.tile([128, R], F32R, tag="zsq")
                    nc.vector.tensor_mul(zsq[:], zc[:], zc[:])
                    zcs.append(zc)
                    zsqs.append(zsq)
                st = ps_st.tile([128, R], F32, tag="st")
                for ot in range(OT):
                    nc.tensor.matmul(
                        st[:],
                        _r(onest[:]),
                        _r(zsqs[ot][:]),
                        start=(ot == 0),
                        stop=(ot == OT - 1),
                        skip_group_check=True,
                    )
                # sqrt(var + eps), var = sumsq / 1024 ; then 1/sqrt on DVE
                sq = p_sq.tile([128, R], F32, tag="sq")
                nc.scalar.activation(
                    sq[:], st[:], AF.Sqrt, bias=epst[:], scale=1.0 / W_HID
                )
                rc = p_rc.tile([128, R], F32, tag="rc")
                nc.vector.reciprocal(rc[:], sq[:])
                for ot in range(OT):
                    u = p_u.tile([128, R], F32, tag="u")
                    nc.vector.scalar_tensor_tensor(
                        u[:], zcs[ot][:], gt[k][:, ot : ot + 1], rc[:],
                        op0=ALU.mult, op1=ALU.mult,
                    )
                    nc.scalar.activation(
                        hn[:, ot, :], u[:], AF.Relu, bias=bet[k][:, ot : ot + 1],
                        scale=1.0,
                    )
                hprev = hn

            # ---- output layer: y = relu(h3 @ Wout + bout)
            zo = ps_st.tile([128, R], F32, tag="st")
            for kt in range(KT):
                nc.tensor.matmul(
                    zo[:1, :],
                    _r(wot[:, kt : kt + 1]),
                    _r(hprev[:, kt, :]),
                    start=(kt == 0),
                    stop=(kt == KT - 1),
                )
            ob = p_ob.tile([1, R], F32, tag="ob")
            nc.scalar.activation(ob[:], zo[:1, :], AF.Relu, bias=bot[:, :], scale=1.0)
            nc.sync.dma_start(out_d[c * R : (c + 1) * R, :], ob[:])

    nc.finalize()
    return nc


def build_nc_fast(rows=ROWS):
    """Fast variant, valid when be1..be3 == 0 and g1..g3 > 0 elementwise.

    Uses fp16 matmul operands (1 cycle/row on the PE vs ~2 for fp32r) and
    defers the LayerNorm scaling: LN is invariant to positive per-row scaling
    of its input, and relu commutes with positive per-row scales, so each
    hidden layer just passes h~ = relu(z_c * g) forward unnormalized. The
    cumulative squared scale follows d2_k = m~_k + eps * d2_{k-1} (m~_k =
    weighted mean of z~^2 via a (1/(1024 g^2))-stationary matmul), and a
    single rsqrt per chunk rescales the output-layer logits.
    """
    nchunk = rows // R
    F16 = mybir.dt.float16
    nc = bacc.Bacc()

    x_d = nc.dram_tensor("x", [rows, D_IN], F16, kind="ExternalInput")
    w0_d = nc.dram_tensor("w0s", [128, KT0 * OT * 128], F16, kind="ExternalInput")
    wh_d = [
        nc.dram_tensor(f"w{k}s", [128, KT * OT * 128], F16, kind="ExternalInput")
        for k in (1, 2, 3)
    ]
    sw_d = [
        nc.dram_tensor(f"sw{k}s", [128, OT * 128], F16, kind="ExternalInput")
        for k in (1, 2, 3)
    ]
    wo_d = nc.dram_tensor("wos", [128, KT], F16, kind="ExternalInput")
    b0_d = nc.dram_tensor("b0s", [128, OT], F32, kind="ExternalInput")
    bo_d = nc.dram_tensor("bo", [1, 1], F32, kind="ExternalInput")
    out_d = nc.dram_tensor("out", [rows, 1], F32, kind="ExternalOutput")

    with tile.TileContext(nc) as tc, ExitStack() as ctx:
        const = ctx.enter_context(tc.tile_pool(name="const", bufs=1))
        p_xt = ctx.enter_context(tc.tile_pool(name="xt", bufs=8))
        p_h = ctx.enter_context(tc.tile_pool(name="h", bufs=3))
        p_zsq = ctx.enter_context(tc.tile_pool(name="zsq", bufs=3))
        p_d2 = ctx.enter_context(tc.tile_pool(name="d2", bufs=4))
        p_s = ctx.enter_context(tc.tile_pool(name="s", bufs=2))
        p_ob = ctx.enter_context(tc.tile_pool(name="ob", bufs=3))
        ps_z = ctx.enter_context(tc.tile_pool(name="psz", bufs=4, space="PSUM"))
        ps_st = ctx.enter_context(tc.tile_pool(name="psst", bufs=3, space="PSUM"))
        ps_zo = ctx.enter_context(tc.tile_pool(name="pszo", bufs=1, space="PSUM"))

        # --- resident constants ---
        w0t = const.tile([128, KT0 * OT * 128], F16)
        nc.scalar.dma_start(w0t[:], w0_d[:])
        wht = []
        swt = []
        for k in range(3):
            t = const.tile([128, KT * OT * 128], F16, tag=f"w{k + 1}t")
            nc.scalar.dma_start(t[:], wh_d[k][:])
            wht.append(t)
            t = const.tile([128, OT, 128], F16, tag=f"sw{k + 1}t")
            nc.scalar.dma_start(t[:], sw_d[k][:])
            swt.append(t)
        wot = const.tile([128, KT], F16)
        nc.scalar.dma_start(wot[:], wo_d[:])
        b0t = const.tile([128, OT], F32)
        nc.scalar.dma_start(b0t[:], b0_d[:])
        bot = const.tile([1, 1], F32)
        nc.scalar.dma_start(bot[:], bo_d[:])
        epst = const.tile([128, 1], F32)
        nc.vector.memset(epst[:], LN_EPS)
        zerot = const.tile([128, 1], F32)
        nc.vector.memset(zerot[:], 0.0)

        w0v = w0t[:].rearrange("p (kt ot m) -> p kt ot m", kt=KT0, ot=OT)
        whv = [t[:].rearrange("p (kt ot m) -> p kt ot m", kt=KT, ot=OT) for t in wht]

        pend = None  # delayed stats matmul: (st, sw_tile, ot, zsq)
        d2_box = [None]  # running d2 tile within the current chunk

        def flush_stats(nc):
            # Emit the one-tile-delayed stats matmul; on the layer's last tile
            # also emit that layer's d2 recursion step (d2 = m~ + eps*d2_prev)
            # so only sqrt/rsqrt/rescale remain after the output-layer matmuls.
            nonlocal pend
            if pend is None:
                return
            st_, sw_, ot_, zsq_ = pend
            nc.tensor.matmul(
                st_[:], sw_[:, ot_, :], zsq_[:],
                start=(ot_ == 0), stop=(ot_ == OT - 1),
                skip_group_check=True,
            )
            pend = None
            if ot_ == OT - 1:
                d2n = p_d2.tile([128, R], F32, tag="d2")
                if d2_box[0] is None:
                    nc.scalar.activation(
                        d2n[:], st_[:], AF.Identity, bias=epst[:], scale=1.0
                    )
                else:
                    nc.vector.scalar_tensor_tensor(
                        d2n[:], d2_box[0][:], LN_EPS, st_[:],
                        op0=ALU.mult, op1=ALU.add,
                    )
                d2_box[0] = d2n

        for c in range(nchunk):
            d2_box[0] = None
            # ---- X chunk straight to feature-major xT[feat, rows] via xbar DMA
            xts = []
            for ft in range(KT0):
                t = p_xt.tile([128, R], F16, tag="xt")
                nc.sync.dma_start(
                    t[:],
                    x_d[c * R : (c + 1) * R, ft * 128 : (ft + 1) * 128],
                    transpose=True,
                )
                xts.append(t)

            # ---- layer 0: h0 = relu(X @ W0 + b0)   (relu on DVE)
            hprev = p_h.tile([128, KT, R], F16, tag="h")
            for ot in range(OT):
                z = ps_z.tile([128, R], F32, tag="z")
                for kt in range(KT0):
                    nc.tensor.matmul(
                        z[:], w0v[:, kt, ot, :], xts[kt][:],
                        start=(kt == 0), stop=(kt == KT0 - 1),
                    )
                flush_stats(nc)
                nc.scalar.activation(
                    hprev[:, ot, :], z[:], AF.Relu, bias=b0t[:, ot : ot + 1],
                    scale=1.0,
                )

            # ---- hidden layers: h~ = relu(h~prev @ (Wc*g)); m~ accumulated on PE
            # stats matmuls are emitted one output-tile late so they never make
            # the PE (strict FIFO) wait on the ACT square of the current tile.
            for k in range(3):
                hn = p_h.tile([128, KT, R], F16, tag="h")
                st = ps_st.tile([128, R], F32, tag="st")
                for ot in range(OT):
                    z = ps_z.tile([128, R], F32, tag="z")
                    for kt in range(KT):
                        nc.tensor.matmul(
                            z[:], whv[k][:, kt, ot, :], hprev[:, kt, :],
                            start=(kt == 0), stop=(kt == KT - 1),
                        )
                    flush_stats(nc)
                    zsq = p_zsq.tile([128, R], F16, tag="zsq")
                    nc.scalar.activation(
                        zsq[:], z[:], AF.Square, bias=zerot[:], scale=1.0
                    )
                    nc.scalar.activation(
                        hn[:, ot, :], z[:], AF.Relu, bias=zerot[:], scale=1.0
                    )
                    pend = (st, swt[k], ot, zsq)
                hprev = hn

            # ---- output layer matmuls (flush last stats after the first ones)
            zo = ps_zo.tile([128, R], F32, tag="zo")
            for kt in range(KT):
                nc.tensor.matmul(
                    zo[:1, :], wot[:, kt : kt + 1], hprev[:, kt, :],
                    start=(kt == 0), stop=(kt == KT - 1),
                )
                if kt == 0:
                    flush_stats(nc)

            # ---- rescale by rsqrt(d2_3) and relu
            s = p_s.tile([128, R], F32, tag="s")
            nc.scalar.activation(s[:], d2_box[0][:], AF.Sqrt, bias=zerot[:], scale=1.0)
            rs = p_s.tile([128, R], F32, tag="rs")
            nc.vector.reciprocal(rs[:1, :], s[:1, :])
            yv = p_ob.tile([1, R], F32, tag="yv")
            nc.vector.tensor_mul(yv[:], zo[:1, :], rs[:1, :])
            ob = p_ob.tile([1, R], F32, tag="ob")
            nc.vector.tensor_scalar(
                ob[:], yv[:], bot[:, :], 0.0, op0=ALU.add, op1=ALU.max
            )
            nc.sync.dma_start(out_d[c * R : (c + 1) * R, :], ob[:])

    nc.finalize()
    return nc


def _prep_inputs(inputs):
    """Host-side weight repack (float64 intermediates for the centering)."""
    f32 = np.float32

    def center(w):
        w64 = np.asarray(w, np.float64)
        return (w64 - w64.mean(axis=1, keepdims=True)).astype(f32)

    def pack_w(w, kt):
        # [kt*128, 1024] -> sbuf layout [p, kt, ot, m] flattened
        return (
            np.ascontiguousarray(
                np.asarray(w, f32).reshape(kt, 128, OT, 128).transpose(1, 0, 2, 3)
            ).reshape(128, kt * OT * 128)
        )

    def pack_v(v):
        # [1024] -> [128, OT] with [:, ot] = v[ot*128:(ot+1)*128]
        return np.ascontiguousarray(np.asarray(v, f32).reshape(OT, 128).T)

    common = {
        "w0s": pack_w(inputs["W0"], KT0),
        "w1s": pack_w(center(inputs["W1"]), KT),
        "w2s": pack_w(center(inputs["W2"]), KT),
        "w3s": pack_w(center(inputs["W3"]), KT),
        "wos": np.ascontiguousarray(np.asarray(inputs["Wout"], f32).reshape(KT, 128).T),
        "b0s": pack_v(inputs["b0"]),
        "g1s": pack_v(inputs["g1"]),
        "be1s": pack_v(inputs["be1"]),
        "g2s": pack_v(inputs["g2"]),
        "be2s": pack_v(inputs["be2"]),
        "g3s": pack_v(inputs["g3"]),
        "be3s": pack_v(inputs["be3"]),
        "bo": np.asarray(inputs["bout"], f32).reshape(1, 1),
        "ident": np.eye(128, dtype=f32),
        "onesm": np.ones((128, 128), dtype=f32),
    }
    return common


def _prep_inputs_fast(inputs):
    """Host prep for the fast (be==0, g>0) variant: fp16 weights, g folded
    into the hidden weights, 1/(1024 g^2) folded into the stats stationary."""
    f16 = np.float16

    def center(w):
        w64 = np.asarray(w, np.float64)
        return w64 - w64.mean(axis=1, keepdims=True)

    def pack_w(w64, kt):
        return np.ascontiguousarray(
            w64.reshape(kt, 128, OT, 128).transpose(1, 0, 2, 3)
        ).reshape(128, kt * OT * 128).astype(f16)

    def stats_w(g):
        g64 = np.asarray(g, np.float64)
        vals = (1.0 / (W_HID * g64 * g64)).reshape(OT, 128)  # [ot, p]
        return np.ascontiguousarray(
            np.broadcast_to(vals.T[:, :, None], (128, OT, 128))
        ).reshape(128, OT * 128).astype(f16)

    common = {
        "w0s": pack_w(np.asarray(inputs["W0"], np.float64), KT0),
        "wos": np.ascontiguousarray(
            np.asarray(inputs["Wout"], np.float64).reshape(KT, 128).T
        ).astype(f16),
        "b0s": np.ascontiguousarray(
            np.asarray(inputs["b0"], np.float32).reshape(OT, 128).T
        ),
        "bo": np.asarray(inputs["bout"], np.float32).reshape(1, 1),
    }
    for k in (1, 2, 3):
        g64 = np.asarray(inputs[f"g{k}"], np.float64)
        common[f"w{k}s"] = pack_w(center(inputs[f"W{k}"]) * g64[None, :], KT)
        common[f"sw{k}s"] = stats_w(g64)
    return common


_NC_CACHE = {}


def _get_nc(fast):
    key = "fast" if fast else "general"
    if key not in _NC_CACHE:
        _NC_CACHE[key] = build_nc_fast() if fast else build_nc()
    return _NC_CACHE[key]


def _is_fast_ok(inputs):
    return all(
        np.all(np.asarray(inputs[f"be{k}"]) == 0)
        and np.all(np.asarray(inputs[f"g{k}"]) > 0)
        for k in (1, 2, 3)
    )


def _run(inputs, trace=False):
    fast = _is_fast_ok(inputs)
    common = _prep_inputs_fast(inputs) if fast else _prep_inputs(inputs)
    xdt = np.float16 if fast else np.float32
    x = np.ascontiguousarray(np.asarray(inputs["descriptors"], np.float32).astype(xdt))
    shards = x.reshape(N_CORES, ROWS, D_IN)
    in_maps = [dict(common, x=np.ascontiguousarray(shards[i])) for i in range(N_CORES)]
    nc = _get_nc(fast)
    res = run_bass_kernel_spmd(nc, in_maps, core_ids=list(range(N_CORES)), trace=trace)
    out = np.concatenate([res.results[i]["out"] for i in range(N_CORES)], axis=0)
    return out.astype(np.float32), res


def kernel(**inputs):
    out, _ = _run(inputs, trace=False)
    return out


def kernel_traced(**inputs):
    out, res = _run(inputs, trace=True)
    return out, res


# revision 17
# speedup vs baseline: 1.7118x; 1.0846x over previous
"""Trainium2 Bass kernel for nn_CoreRelu_83863531422003 (5-layer MLP).

Network (per reference):
    h0 = relu(X @ W0 + b0)                      X:[N,512] W0:[512,1024]
    hk = relu(LN(h_{k-1} @ Wk) * gk + bek)      Wk:[1024,1024], k=1..3
    y  = relu(h3 @ Wout + bout)                 Wout:[1024,1]

Sharding: data-parallel over rows across 8 NeuronCores (8192 rows/core),
weights replicated. No communication.

On-chip layout ("option B", feature-major): activations live transposed in
SBUF as [feat(partition), rows(free)]; weights are the stationary matmul
operand so layer outputs stay feature-major and no per-layer transposes are
needed. Only X is transposed once (PE transpose) on entry.

LayerNorm: host pre-centers hidden weights (W_c = W - W.mean(axis=1)) so
z = h @ W_c is exactly mean-free; variance = sumsq(z)/1024, computed with a
ones[128x128]-stationary matmul over z^2 which also replicates the per-row
sum across all 128 partitions (needed for the feature-major apply).

Matmuls run as float32r (1 cycle/row for moving free dim >=256 vs 4 for
plain fp32).
"""

import numpy as np
from contextlib import ExitStack

import concourse.bass as bass
import concourse.bacc as bacc
import concourse.tile as tile
from concourse import mybir
from concourse.bass_utils import run_bass_kernel_spmd

N_CORES = 8
N_FULL = 65536
D_IN = 512
W_HID = 1024
ROWS = N_FULL // N_CORES      # 8192 rows per core
R = 512                       # rows per chunk (one PSUM bank of fp32)
NCHUNK = ROWS // R            # 16
KT0 = D_IN // 128             # 4 k-tiles for layer 0
KT = W_HID // 128             # 8 k-tiles for hidden layers
OT = W_HID // 128             # 8 output-feature tiles
LN_EPS = 1e-6

F32 = mybir.dt.float32
F32R = mybir.dt.float32r
MM_DT = F32R     # flip to F32 if fp32r numerics prove too loose

AF = mybir.ActivationFunctionType
ALU = mybir.AluOpType


def _r(ap):
    """Bitcast an AP to the matmul dtype."""
    if MM_DT == F32:
        return ap
    return ap.bitcast(MM_DT)


def build_nc(rows=ROWS):
    nchunk = rows // R
    nc = bacc.Bacc()

    x_d = nc.dram_tensor("x", [rows, D_IN], F32R, kind="ExternalInput")
    w0_d = nc.dram_tensor("w0s", [128, KT0 * OT * 128], F32R, kind="ExternalInput")
    wh_d = [
        nc.dram_tensor(f"w{k}s", [128, KT * OT * 128], F32R, kind="ExternalInput")
        for k in (1, 2, 3)
    ]
    wo_d = nc.dram_tensor("wos", [128, KT], F32R, kind="ExternalInput")
    b0_d = nc.dram_tensor("b0s", [128, OT], F32, kind="ExternalInput")
    g_d = [nc.dram_tensor(f"g{k}s", [128, OT], F32, kind="ExternalInput") for k in (1, 2, 3)]
    be_d = [nc.dram_tensor(f"be{k}s", [128, OT], F32, kind="ExternalInput") for k in (1, 2, 3)]
    bo_d = nc.dram_tensor("bo", [1, 1], F32, kind="ExternalInput")
    out_d = nc.dram_tensor("out", [rows, 1], F32, kind="ExternalOutput")

    ident_d = nc.dram_tensor("ident", [128, 128], F32R, kind="ExternalInput")
    ones_d = nc.dram_tensor("onesm", [128, 128], F32R, kind="ExternalInput")

    with tile.TileContext(nc) as tc, ExitStack() as ctx:
        const = ctx.enter_context(tc.tile_pool(name="const", bufs=1))
        p_xin = ctx.enter_context(tc.tile_pool(name="xin", bufs=4))
        p_xt = ctx.enter_context(tc.tile_pool(name="xt", bufs=2))
        p_h = ctx.enter_context(tc.tile_pool(name="h", bufs=2))
        p_zc = ctx.enter_context(tc.tile_pool(name="zc", bufs=8))
        p_zsq = ctx.enter_context(tc.tile_pool(name="zsq", bufs=2))
        p_u = ctx.enter_context(tc.tile_pool(name="u", bufs=2))
        p_sq = ctx.enter_context(tc.tile_pool(name="sq", bufs=2))
        p_rc = ctx.enter_context(tc.tile_pool(name="rc", bufs=2))
        p_ob = ctx.enter_context(tc.tile_pool(name="ob", bufs=2))
        ps_z = ctx.enter_context(tc.tile_pool(name="psz", bufs=5, space="PSUM"))
        ps_tp = ctx.enter_context(tc.tile_pool(name="pstp", bufs=2, space="PSUM"))
        ps_st = ctx.enter_context(tc.tile_pool(name="psst", bufs=1, space="PSUM"))

        # --- resident constants ---
        w0t = const.tile([128, KT0 * OT * 128], F32R)
        nc.sync.dma_start(w0t[:], w0_d[:])
        wht = []
        for k in range(3):
            t = const.tile([128, KT * OT * 128], F32R, tag=f"w{k + 1}t")
            nc.sync.dma_start(t[:], wh_d[k][:])
            wht.append(t)
        wot = const.tile([128, KT], F32R)
        nc.sync.dma_start(wot[:], wo_d[:])
        b0t = const.tile([128, OT], F32)
        nc.sync.dma_start(b0t[:], b0_d[:])
        gt, bet = [], []
        for k in range(3):
            g = const.tile([128, OT], F32, tag=f"g{k + 1}t")
            nc.sync.dma_start(g[:], g_d[k][:])
            gt.append(g)
            b = const.tile([128, OT], F32, tag=f"be{k + 1}t")
            nc.sync.dma_start(b[:], be_d[k][:])
            bet.append(b)
        bot = const.tile([1, 1], F32)
        nc.sync.dma_start(bot[:], bo_d[:])
        idt = const.tile([128, 128], F32R)
        nc.sync.dma_start(idt[:], ident_d[:])
        onest = const.tile([128, 128], F32R)
        nc.sync.dma_start(onest[:], ones_d[:])
        epst = const.tile([128, 1], F32)
        nc.vector.memset(epst[:], LN_EPS)

        w0v = w0t[:].rearrange("p (kt ot m) -> p kt ot m", kt=KT0, ot=OT)
        whv = [t[:].rearrange("p (kt ot m) -> p kt ot m", kt=KT, ot=OT) for t in wht]

        for c in range(nchunk):
            # ---- load X chunk and transpose to feature-major xT[feat, rows]
            xin = []
            for rg in range(4):
                t = p_xin.tile([128, D_IN], F32R, tag="xin")
                nc.sync.dma_start(t[:], x_d[c * R + rg * 128 : c * R + (rg + 1) * 128, :])
                xin.append(t)
            xt = p_xt.tile([128, KT0, R], F32R, tag="xt")
            for rg in range(4):
                for ft in range(KT0):
                    tp = ps_tp.tile([128, 128], F32, tag="tp")
                    nc.tensor.transpose(
                        _r(tp[:]), _r(xin[rg][:, ft * 128 : (ft + 1) * 128]), _r(idt[:])
                    )
                    nc.scalar.copy(xt[:, ft, rg * 128 : (rg + 1) * 128], tp[:])

            # ---- layer 0: h0 = relu(X @ W0 + b0)
            hprev = p_h.tile([128, KT, R], F32R, tag="h")
            for ot in range(OT):
                z = ps_z.tile([128, R], F32, tag="z")
                for kt in range(KT0):
                    nc.tensor.matmul(
                        z[:],
                        _r(w0v[:, kt, ot, :]),
                        _r(xt[:, kt, :]),
                        start=(kt == 0),
                        stop=(kt == KT0 - 1),
                    )
                nc.scalar.activation(
                    hprev[:, ot, :], z[:], AF.Relu, bias=b0t[:, ot : ot + 1], scale=1.0
                )

            # ---- hidden layers 1..3: h = relu(LN(h @ Wc) * g + be)
            for k in range(3):
                hn = p_h.tile([128, KT, R], F32R, tag="h")
                zcs = []
                zsqs = []
                for ot in range(OT):
                    z = ps_z.tile([128, R], F32, tag="z")
                    for kt in range(KT):
                        nc.tensor.matmul(
                            z[:],
                            _r(whv[k][:, kt, ot, :]),
                            _r(hprev[:, kt, :]),
                            start=(kt == 0),
                            stop=(kt == KT - 1),
                        )
                    zc = p_zc.tile([128, R], F32, tag="zc")
                    nc.scalar.copy(zc[:], z[:])
                    zsq = p_zsq.tile([128, R], F32R, tag="zsq")
                    nc.vector.tensor_mul(zsq[:], zc[:], zc[:])
                    zcs.append(zc)
                    zsqs.append(zsq)
                st = ps_st.tile([128, R], F32, tag="st")
                for ot in range(OT):
                    nc.tensor.matmul(
                        st[:],
                        _r(onest[:]),
                        _r(zsqs[ot][:]),
                        start=(ot == 0),
                        stop=(ot == OT - 1),
                        skip_group_check=True,
                    )
                # sqrt(var + eps), var = sumsq / 1024 ; then 1/sqrt on DVE
                sq = p_sq.tile([128, R], F32, tag="sq")
                nc.scalar.activation(
                    sq[:], st[:], AF.Sqrt, bias=epst[:], scale=1.0 / W_HID
                )
                rc = p_rc.tile([128, R], F32, tag="rc")
                nc.vector.reciprocal(rc[:], sq[:])
                for ot in range(OT):
                    u = p_u.tile([128, R], F32, tag="u")
                    nc.vector.scalar_tensor_tensor(
                        u[:], zcs[ot][:], gt[k][:, ot : ot + 1], rc[:],
                        op0=ALU.mult, op1=ALU.mult,
                    )
                    nc.scalar.activation(
                        hn[:, ot, :], u[:], AF.Relu, bias=bet[k][:, ot : ot + 1],
                        scale=1.0,
                    )
                hprev = hn

            # ---- output layer: y = relu(h3 @ Wout + bout)
            zo = ps_st.tile([128, R], F32, tag="st")
            for kt in range(KT):
                nc.tensor.matmul(
                    zo[:1, :],
                    _r(wot[:, kt : kt + 1]),
                    _r(hprev[:, kt, :]),
                    start=(kt == 0),
                    stop=(kt == KT - 1),
                )
            ob = p_ob.tile([1, R], F32, tag="ob")
            nc.scalar.activation(ob[:], zo[:1, :], AF.Relu, bias=bot[:, :], scale=1.0)
            nc.sync.dma_start(out_d[c * R : (c + 1) * R, :], ob[:])

    nc.finalize()
    return nc


def build_nc_fast(rows=ROWS):
    """Fast variant, valid when be1..be3 == 0 and g1..g3 > 0 elementwise.

    Uses fp16 matmul operands (1 cycle/row on the PE vs ~2 for fp32r) and
    defers the LayerNorm scaling: LN is invariant to positive per-row scaling
    of its input, and relu commutes with positive per-row scales, so each
    hidden layer just passes h~ = relu(z_c * g) forward unnormalized. The
    cumulative squared scale follows d2_k = m~_k + eps * d2_{k-1} (m~_k =
    weighted mean of z~^2 via a (1/(1024 g^2))-stationary matmul), and a
    single rsqrt per chunk rescales the output-layer logits.
    """
    nchunk = rows // R
    F16 = mybir.dt.float16
    nc = bacc.Bacc()

    x_d = nc.dram_tensor("x", [rows, D_IN], F16, kind="ExternalInput")
    w0_d = nc.dram_tensor("w0s", [128, KT0 * OT * 128], F16, kind="ExternalInput")
    wh_d = [
        nc.dram_tensor(f"w{k}s", [128, KT * OT * 128], F16, kind="ExternalInput")
        for k in (1, 2, 3)
    ]
    sg_d = [
        nc.dram_tensor(f"sg{k}s", [128, OT], F32, kind="ExternalInput")
        for k in (1, 2, 3)
    ]
    ones_d = nc.dram_tensor("onesm", [128, 128], F16, kind="ExternalInput")
    wo_d = nc.dram_tensor("wos", [128, KT], F16, kind="ExternalInput")
    b0_d = nc.dram_tensor("b0s", [128, OT], F32, kind="ExternalInput")
    bo_d = nc.dram_tensor("bo", [1, 1], F32, kind="ExternalInput")
    out_d = nc.dram_tensor("out", [rows, 1], F32, kind="ExternalOutput")

    with tile.TileContext(nc) as tc, ExitStack() as ctx:
        const = ctx.enter_context(tc.tile_pool(name="const", bufs=1))
        p_xt = ctx.enter_context(tc.tile_pool(name="xt", bufs=8))
        p_h = ctx.enter_context(tc.tile_pool(name="h", bufs=3))
        p_zsq = ctx.enter_context(tc.tile_pool(name="zsq", bufs=3))
        p_d2 = ctx.enter_context(tc.tile_pool(name="d2", bufs=4))
        p_s = ctx.enter_context(tc.tile_pool(name="s", bufs=2))
        p_ob = ctx.enter_context(tc.tile_pool(name="ob", bufs=3))
        ps_z = ctx.enter_context(tc.tile_pool(name="psz", bufs=4, space="PSUM"))
        ps_st = ctx.enter_context(tc.tile_pool(name="psst", bufs=3, space="PSUM"))
        ps_zo = ctx.enter_context(tc.tile_pool(name="pszo", bufs=1, space="PSUM"))

        # --- resident constants (w0 + small vectors first: L0 needs them) ---
        w0t = const.tile([128, KT0 * OT * 128], F16)
        nc.scalar.dma_start(w0t[:], w0_d[:])
        b0t = const.tile([128, OT], F32)
        nc.scalar.dma_start(b0t[:], b0_d[:])
        bot = const.tile([1, 1], F32)
        nc.scalar.dma_start(bot[:], bo_d[:])
        onest = const.tile([128, 128], F16)
        nc.scalar.dma_start(onest[:], ones_d[:])
        sgt = []
        for k in range(3):
            t = const.tile([128, OT], F32, tag=f"sg{k + 1}t")
            nc.scalar.dma_start(t[:], sg_d[k][:])
            sgt.append(t)
        wht = []
        for k in range(3):
            t = const.tile([128, KT * OT * 128], F16, tag=f"w{k + 1}t")
            nc.scalar.dma_start(t[:], wh_d[k][:])
            wht.append(t)
        wot = const.tile([128, KT], F16)
        nc.scalar.dma_start(wot[:], wo_d[:])
        epst = const.tile([128, 1], F32)
        nc.vector.memset(epst[:], LN_EPS)
        zerot = const.tile([128, 1], F32)
        nc.vector.memset(zerot[:], 0.0)

        w0v = w0t[:].rearrange("p (kt ot m) -> p kt ot m", kt=KT0, ot=OT)
        whv = [t[:].rearrange("p (kt ot m) -> p kt ot m", kt=KT, ot=OT) for t in wht]

        pend = None  # delayed stats matmul: (st, acc, first, last)
        d2_box = [None]  # running d2 tile within the current chunk

        def flush_stats(nc):
            # Emit the delayed stats matmul (ones-stationary over the folded,
            # pre-weighted z^2 accumulator); on the layer's last flush also
            # emit that layer's d2 recursion step (d2 = m~ + eps*d2_prev) so
            # only sqrt/rsqrt/rescale remain after the output-layer matmuls.
            nonlocal pend
            if pend is None:
                return
            st_, acc_, first_, last_ = pend
            nc.tensor.matmul(
                st_[:], onest[:], acc_[:],
                start=first_, stop=last_,
                skip_group_check=True,
            )
            pend = None
            if last_:
                d2n = p_d2.tile([128, R], F32, tag="d2")
                if d2_box[0] is None:
                    nc.scalar.activation(
                        d2n[:], st_[:], AF.Identity, bias=epst[:], scale=1.0
                    )
                else:
                    nc.vector.scalar_tensor_tensor(
                        d2n[:], d2_box[0][:], LN_EPS, st_[:],
                        op0=ALU.mult, op1=ALU.add,
                    )
                d2_box[0] = d2n

        for c in range(nchunk):
            d2_box[0] = None
            # ---- X chunk straight to feature-major xT[feat, rows] via xbar DMA
            xts = []
            for ft in range(KT0):
                t = p_xt.tile([128, R], F16, tag="xt")
                nc.sync.dma_start(
                    t[:],
                    x_d[c * R : (c + 1) * R, ft * 128 : (ft + 1) * 128],
                    transpose=True,
                )
                xts.append(t)

            # ---- layer 0: h0 = relu(X @ W0 + b0)   (relu on DVE)
            hprev = p_h.tile([128, KT, R], F16, tag="h")
            for ot in range(OT):
                z = ps_z.tile([128, R], F32, tag="z")
                for kt in range(KT0):
                    nc.tensor.matmul(
                        z[:], w0v[:, kt, ot, :], xts[kt][:],
                        start=(kt == 0), stop=(kt == KT0 - 1),
                    )
                flush_stats(nc)
                nc.scalar.activation(
                    hprev[:, ot, :], z[:], AF.Relu, bias=b0t[:, ot : ot + 1],
                    scale=1.0,
                )

            # ---- hidden layers: h~ = relu(h~prev @ (Wc*g)); m~ accumulated on PE
            # stats matmuls are emitted one output-tile late so they never make
            # the PE (strict FIFO) wait on the ACT square of the current tile.
            for k in range(3):
                hn = p_h.tile([128, KT, R], F16, tag="h")
                st = ps_st.tile([128, R], F32, tag="st")
                for ot in range(OT):
                    z = ps_z.tile([128, R], F32, tag="z")
                    for kt in range(KT):
                        nc.tensor.matmul(
                            z[:], whv[k][:, kt, ot, :], hprev[:, kt, :],
                            start=(kt == 0), stop=(kt == KT - 1),
                        )
                    flush_stats(nc)
                    zsq = p_zsq.tile([128, R], F16, tag="zsq")
                    nc.scalar.activation(
                        zsq[:], z[:], AF.Square, bias=zerot[:],
                        scale=sgt[k][:, ot : ot + 1],
                    )
                    nc.scalar.activation(
                        hn[:, ot, :], z[:], AF.Relu, bias=zerot[:], scale=1.0
                    )
                    if ot % 4 == 0:
                        acc = zsq
                    else:
                        nc.vector.tensor_add(acc[:], acc[:], zsq[:])
                    if ot % 4 == 3:
                        pend = (st, acc, ot == 3, ot == OT - 1)
                hprev = hn

            # ---- output layer matmuls (flush last stats after the first ones)
            zo = ps_zo.tile([128, R], F32, tag="zo")
            for kt in range(KT):
                nc.tensor.matmul(
                    zo[:1, :], wot[:, kt : kt + 1], hprev[:, kt, :],
                    start=(kt == 0), stop=(kt == KT - 1),
                )
                if kt == 0:
                    flush_stats(nc)

            # ---- rescale by rsqrt(d2_3) and relu
            s = p_s.tile([128, R], F32, tag="s")
            nc.scalar.activation(s[:], d2_box[0][:], AF.Sqrt, bias=zerot[:], scale=1.0)
            rs = p_s.tile([128, R], F32, tag="rs")
            nc.vector.reciprocal(rs[:1, :], s[:1, :])
            yv = p_ob.tile([1, R], F32, tag="yv")
            nc.vector.tensor_mul(yv[:], zo[:1, :], rs[:1, :])
            ob = p_ob.tile([1, R], F32, tag="ob")
            nc.vector.tensor_scalar(
                ob[:], yv[:], bot[:, :], 0.0, op0=ALU.add, op1=ALU.max
            )
            nc.sync.dma_start(out_d[c * R : (c + 1) * R, :], ob[:])

    nc.finalize()
    return nc


def _prep_inputs(inputs):
    """Host-side weight repack (float64 intermediates for the centering)."""
    f32 = np.float32

    def center(w):
        w64 = np.asarray(w, np.float64)
        return (w64 - w64.mean(axis=1, keepdims=True)).astype(f32)

    def pack_w(w, kt):
        # [kt*128, 1024] -> sbuf layout [p, kt, ot, m] flattened
        return (
            np.ascontiguousarray(
                np.asarray(w, f32).reshape(kt, 128, OT, 128).transpose(1, 0, 2, 3)
            ).reshape(128, kt * OT * 128)
        )

    def pack_v(v):
        # [1024] -> [128, OT] with [:, ot] = v[ot*128:(ot+1)*128]
        return np.ascontiguousarray(np.asarray(v, f32).reshape(OT, 128).T)

    common = {
        "w0s": pack_w(inputs["W0"], KT0),
        "w1s": pack_w(center(inputs["W1"]), KT),
        "w2s": pack_w(center(inputs["W2"]), KT),
        "w3s": pack_w(center(inputs["W3"]), KT),
        "wos": np.ascontiguousarray(np.asarray(inputs["Wout"], f32).reshape(KT, 128).T),
        "b0s": pack_v(inputs["b0"]),
        "g1s": pack_v(inputs["g1"]),
        "be1s": pack_v(inputs["be1"]),
        "g2s": pack_v(inputs["g2"]),
        "be2s": pack_v(inputs["be2"]),
        "g3s": pack_v(inputs["g3"]),
        "be3s": pack_v(inputs["be3"]),
        "bo": np.asarray(inputs["bout"], f32).reshape(1, 1),
        "ident": np.eye(128, dtype=f32),
        "onesm": np.ones((128, 128), dtype=f32),
    }
    return common


def _prep_inputs_fast(inputs):
    """Host prep for the fast (be==0, g>0) variant: fp16 weights, g folded
    into the hidden weights, 1/(1024 g^2) folded into the stats stationary."""
    f16 = np.float16

    def center(w):
        w64 = np.asarray(w, np.float64)
        return w64 - w64.mean(axis=1, keepdims=True)

    def pack_w(w64, kt):
        return np.ascontiguousarray(
            w64.reshape(kt, 128, OT, 128).transpose(1, 0, 2, 3)
        ).reshape(128, kt * OT * 128).astype(f16)

    def stats_g(g):
        # per-feature ACT-square scale: Square(z*s) = z^2/(1024 g^2)
        g64 = np.asarray(g, np.float64)
        vals = (1.0 / (np.sqrt(W_HID) * g64)).reshape(OT, 128)  # [ot, p]
        return np.ascontiguousarray(vals.T).astype(np.float32)

    common = {
        "w0s": pack_w(np.asarray(inputs["W0"], np.float64), KT0),
        "wos": np.ascontiguousarray(
            np.asarray(inputs["Wout"], np.float64).reshape(KT, 128).T
        ).astype(f16),
        "b0s": np.ascontiguousarray(
            np.asarray(inputs["b0"], np.float32).reshape(OT, 128).T
        ),
        "bo": np.asarray(inputs["bout"], np.float32).reshape(1, 1),
        "onesm": np.ones((128, 128), dtype=f16),
    }
    for k in (1, 2, 3):
        g64 = np.asarray(inputs[f"g{k}"], np.float64)
        common[f"w{k}s"] = pack_w(center(inputs[f"W{k}"]) * g64[None, :], KT)
        common[f"sg{k}s"] = stats_g(g64)
    return common


_NC_CACHE = {}


def _get_nc(fast):
    key = "fast" if fast else "general"
    if key not in _NC_CACHE:
        _NC_CACHE[key] = build_nc_fast() if fast else build_nc()
    return _NC_CACHE[key]


def _is_fast_ok(inputs):
    return all(
        np.all(np.asarray(inputs[f"be{k}"]) == 0)
        and np.all(np.asarray(inputs[f"g{k}"]) > 0)
        for k in (1, 2, 3)
    )


def _run(inputs, trace=False):
    fast = _is_fast_ok(inputs)
    common = _prep_inputs_fast(inputs) if fast else _prep_inputs(inputs)
    xdt = np.float16 if fast else np.float32
    x = np.ascontiguousarray(np.asarray(inputs["descriptors"], np.float32).astype(xdt))
    shards = x.reshape(N_CORES, ROWS, D_IN)
    in_maps = [dict(common, x=np.ascontiguousarray(shards[i])) for i in range(N_CORES)]
    nc = _get_nc(fast)
    res = run_bass_kernel_spmd(nc, in_maps, core_ids=list(range(N_CORES)), trace=trace)
    out = np.concatenate([res.results[i]["out"] for i in range(N_CORES)], axis=0)
    return out.astype(np.float32), res


def kernel(**inputs):
    out, _ = _run(inputs, trace=False)
    return out


def kernel_traced(**inputs):
    out, res = _run(inputs, trace=True)
    return out, res
